# revision 56
# baseline (speedup 1.0000x reference)
"""GQA attention forward (B=2, S=2048, D=2048, 16 q heads / 4 kv heads, RoPE,
causal) on 8 Trainium2 NeuronCores.

Sharding: core c <-> (batch b = c//4, kv-group g = c%4). Each core computes its
4 query heads + 1 kv head end-to-end, including its row-shard of wo; the host
sums the 4 wo-partials per batch (the "all-reduce after wo" of the tensor
parallel scheme, done at gather time).

Layout tricks:
  - x is passed transposed (d-major) so every matmul contraction dim lands on
    SBUF partitions.
  - wq/wk columns are permuted per head (even dims -> partitions 0..63, odd ->
    64..127) so RoPE becomes plain elementwise DVE math on partition halves.
    The permutation cancels in q.k dot products.
  - all matmuls run in bf16 (fp8 DoubleRow measured exactly 2x on HW, so
    error-compensated fp8 (3 matmuls per 2 bf16-equivalents) is a net loss);
    accumulation stays fp32 in PSUM.
  - scores are built transposed ([t, s]); the softmax denominator is an
    all-ones-matrix matmul accumulated in PSUM, which lands the denominator
    already broadcast across partitions.
  - deferred-work queues (high-prio: next-block projection chains + v
    transposes; low-prio: wo chains) hold per-matmul micro-ops; the attention
    tile loop stuffs them into the PE slack left by the scalar-engine exp
    pacing (~220ns/tile). Each block's q chains are force-drained during the
    previous attention phase / head 0 so their rope latency (~4us serial
    DVE+gpsimd per chain) hides behind a full head of attention work.
    wo backlog is retained so block 3's large attention phase has stuff work,
    and a small wo reserve bridges the final norm's latency (a PE idle there
    drops the clock p-state and slows the whole wo drain tail ~630ns/matmul).
  - startup: the PE warmup (clock-gate ramp) runs on a memset tile (no DMA
    dependency, first matmul ~7.5us); block-0 x rides sync while wqkv h1 +
    fp16 cos/sin ride scalar, ordered by first use, so the dt-interleaved
    block-0 k/q0/v prolog starts on the first quarter (~13us) and streams at
    DMA arrival pace. x s-blocks 1-3 load s-block-major so block sj+1's
    projections never wait on a later quarter. y writes DMA via the
    otherwise-idle sync engine (scalar issue cost would eat exp headroom).

Measured (8 cores, core-0 profile): 262.6-263.6us; PE busy ~221us of that.
Dead ends measured on HW: fp8 DoubleRow is exactly 2x bf16 per matmul, so
error-compensated fp8 (3 matmuls per 2 bf16-equivalents) is a 1.5x net loss;
plain fp8 fails the 2e-2 gate (5.7e-2); half-width (256) attention segments
double per-op overheads and flip block 3 scalar-bound; AV LOOKAHEAD=5
corrupts numerics (es-pool lifetime); gpsimd cannot access PSUM.
"""

import ml_dtypes
import numpy as np

BF = ml_dtypes.bfloat16
F16 = np.float16
B, S, D = 2, 2048, 2048
N_HEADS, N_KV_HEADS, HD = 16, 4, 128
NH = N_HEADS // N_KV_HEADS  # q heads per core = 4
SB = 512                    # s-block (moving dim per matmul)
NSJ = S // SB               # 4 s-blocks
NT = S // HD                # 16 t-tiles (and d-tiles)
NM = NH + 2                 # 6 projection column-blocks: k, v, q0..q3
H2 = HD // 2
SCALE = 1.0 / np.sqrt(HD).astype(np.float32)

_PROG = None  # built once per process


def _build_program():
    import concourse.bacc as bacc
    import concourse.tile as tile
    from concourse import mybir

    F32 = mybir.dt.float32
    BF16 = mybir.dt.bfloat16
    FP16 = mybir.dt.float16
    Exp = mybir.ActivationFunctionType.Exp

    nc = bacc.Bacc("TRN2", target_bir_lowering=False, debug=False)

    xt_d = nc.declare_dram_parameter("xt", [D, S], BF16, isOutput=False)
    wqkv_d = nc.declare_dram_parameter("wqkv", [D, NM * HD], BF16, isOutput=False)
    wo_d = nc.declare_dram_parameter("wo", [NH * HD, D], BF16, isOutput=False)
    cost_d = nc.declare_dram_parameter("cost", [H2, S], FP16, isOutput=False)
    sint_d = nc.declare_dram_parameter("sint", [H2, S], FP16, isOutput=False)
    tri_d = nc.declare_dram_parameter("tri", [HD, HD], BF16, isOutput=False)
    ident_d = nc.declare_dram_parameter("ident", [HD, HD], BF16, isOutput=False)
    ones_d = nc.declare_dram_parameter("ones", [HD, HD], BF16, isOutput=False)
    y_d = nc.declare_dram_parameter("y", [S, D], BF16, isOutput=True)

    with tile.TileContext(nc) as tc:
        with (
            tc.tile_pool(name="consts", bufs=1) as consts,
            tc.tile_pool(name="persist", bufs=1) as persist,
            tc.tile_pool(name="work", bufs=2) as work,
            tc.tile_pool(name="xts_pool", bufs=1) as xts_pool,
            tc.tile_pool(name="qk_pool", bufs=1) as qk_pool,
            tc.tile_pool(name="es_pool", bufs=1) as es_pool,
            tc.tile_pool(name="ps", bufs=1, space="PSUM") as ps,
        ):
            tri = consts.tile([HD, HD], BF16, tag="tri")
            ident = consts.tile([HD, HD], BF16, tag="ident")
            ones_sb = consts.tile([HD, HD], BF16, tag="ones")
            cost = consts.tile([H2, S], FP16, tag="cost")
            sint = consts.tile([H2, S], FP16, tag="sint")

            wqkv = persist.tile([HD, NT, NM * HD], BF16, tag="wqkv")
            kt = persist.tile([HD, S], BF16, tag="kt")
            v_sb = persist.tile([HD, NT, HD], BF16, tag="v_sb")
            on_sb = persist.tile([HD, NH, S], BF16, tag="on")
            wo_sb = persist.tile([HD, NH, D], BF16, tag="wo")

            xt_r = xt_d[:, :].rearrange("(t p) s -> p t s", p=HD)
            wqkv_r = wqkv_d[:, :].rearrange("(t p) m -> p t m", p=HD)

            # ---- PE warm-up on a memset tile: no DMA dependency, so the
            # clock-gate (HAM) ramp starts as soon as the preamble ends ----
            dmy = consts.tile([HD, SB], BF16, tag="dmy")
            nc.vector.memset(dmy, 0.0)
            ps_warm = ps.tile([HD, SB], F32, tag="s", bufs=3, name="warmup")
            NWARM = 32
            for w in range(NWARM):
                nc.tensor.matmul(
                    out=ps_warm, lhsT=dmy[:, 0:HD], rhs=dmy,
                    start=(w == 0), stop=(w == NWARM - 1),
                )

            # ---- startup DMAs (hwdge queues: sync + scalar; gpsimd swdge
            # only for tiny consts). Ordered by first use so the interleaved
            # block-0 projection prolog can start after the first quarter;
            # two h2 quarters ride sync so all q-head weights land by ~23us ----
            MH = 3 * HD  # first column-half: k, q0, v
            xts_tiles = {}
            xrest_tiles = {}
            # sync: x block-0 quarters in dt order
            for ck in range(4):
                xq = xts_pool.tile(
                    [HD, NT // 4, SB], BF16, tag="xts", bufs=4, name=f"xts_0_{ck}"
                )
                nc.sync.dma_start(out=xq, in_=xt_r[:, ck * 4 : (ck + 1) * 4, 0:SB])
                xts_tiles[(0, ck)] = xq
            # scalar: first wqkv quarter (the PE's first real work), then
            # cos/sin (fp16, needed by rope-k ~20us), then the rest of h1
            nc.scalar.dma_start(
                out=wqkv[:, 0:4, 0:MH], in_=wqkv_r[:, 0:4, 0:MH]
            )
            nc.scalar.dma_start(out=cost, in_=cost_d[:, :])
            nc.scalar.dma_start(out=sint, in_=sint_d[:, :])
            for ck in range(1, 4):
                nc.scalar.dma_start(
                    out=wqkv[:, ck * 4 : (ck + 1) * 4, 0:MH],
                    in_=wqkv_r[:, ck * 4 : (ck + 1) * 4, 0:MH],
                )
            # preload the exp activation table while DMAs stream
            actwarm = work.tile([HD, 1], BF16, tag="actwarm", bufs=1)
            nc.scalar.activation(
                out=actwarm, in_=dmy[:, 0:1],
                func=mybir.ActivationFunctionType.Exp,
            )
            # wqkv second halves (q1..q3 columns): split scalar/sync
            for ck, eng in [(0, nc.scalar), (1, nc.scalar), (2, nc.sync), (3, nc.sync)]:
                eng.dma_start(
                    out=wqkv[:, ck * 4 : (ck + 1) * 4, MH : NM * HD],
                    in_=wqkv_r[:, ck * 4 : (ck + 1) * 4, MH : NM * HD],
                )
            nc.gpsimd.dma_start(out=tri, in_=tri_d[:, :])
            nc.gpsimd.dma_start(out=ident, in_=ident_d[:, :])
            nc.gpsimd.dma_start(out=ones_sb, in_=ones_d[:, :])
            # x s-blocks 1-3, s-block-major so earlier blocks land first
            for sj in range(1, NSJ):
                for ck in range(4):
                    xr = xts_pool.tile(
                        [HD, NT // 4, SB], BF16, tag="xrest", bufs=12,
                        name=f"xrest_{sj}_{ck}",
                    )
                    nc.sync.dma_start(
                        out=xr,
                        in_=xt_r[:, ck * 4 : (ck + 1) * 4, sj * SB : (sj + 1) * SB],
                    )
                    xrest_tiles[(sj, ck)] = xr
            nc.scalar.dma_start(
                out=wo_sb, in_=wo_d[:, :].rearrange("(h p) d -> p h d", p=HD)
            )

            def xq_ap(sj, dt):
                ck, sub = dt // 4, dt % 4
                if sj == 0:
                    return xts_tiles[(0, ck)][:, sub, :]
                return xrest_tiles[(sj, ck)][:, sub, :]

            # ---- deferred-work queues: proj (high prio) and wo (low) ----
            proj_q = []   # ('op', closure) | ('marker', key)
            wo_q = []     # closures
            passed = set()
            q_tiles = {}  # (sj, h) -> tile, filled lazily by rope closures
            vt_pending = {}

            def pop_proj():
                while proj_q:
                    kind, payload = proj_q.pop(0)
                    if kind == "marker":
                        passed.add(payload)
                        continue
                    payload()
                    return True
                return False

            def pop_one(wo_floor=0):
                if pop_proj():
                    return True
                if len(wo_q) > wo_floor:
                    wo_q.pop(0)()
                    return True
                return False

            def drain_until(marker):
                while marker not in passed and proj_q:
                    pop_proj()

            # wqkv column-block order (host-permuted to match consumption):
            # m=0: k, m=1: q0, m=2: v, m=3..5: q1..q3
            def m_to_qhead(m):
                return 0 if m == 1 else m - 2

            def rope_emit(pp, sj, m):
                # rows 0:64 = even dims (xr), 64:128 = odd (xi)
                # out_even = xr*c - xi*s ; out_odd = xr*s + xi*c
                # one PSUM->bf16 copy, then all muls run in DVE 2x mode
                # (fp32-PSUM-input ops cost 717ns vs 335ns for bf16 SBUF)
                s0 = sj * SB
                if m == 0:
                    dst = kt[:, s0 : s0 + SB]
                else:
                    h = m_to_qhead(m)
                    dst = qk_pool.tile(
                        [HD, SB], BF16, tag="qk", bufs=8, name=f"q_{sj}_{h}"
                    )
                    q_tiles[(sj, h)] = dst
                c = cost[:, s0 : s0 + SB]
                sn = sint[:, s0 : s0 + SB]
                ta = work.tile([H2, SB], F32, tag="ropeA")
                tb = work.tile([H2, SB], F32, tag="ropeB")
                nc.vector.tensor_mul(out=ta, in0=pp[0:H2, :], in1=c)
                nc.vector.tensor_mul(out=tb, in0=pp[H2:HD, :], in1=sn)
                nc.gpsimd.tensor_sub(out=dst[0:H2, :], in0=ta, in1=tb)
                tc2 = work.tile([H2, SB], F32, tag="ropeA")
                td = work.tile([H2, SB], F32, tag="ropeB")
                nc.vector.tensor_mul(out=tc2, in0=pp[0:H2, :], in1=sn)
                nc.vector.tensor_mul(out=td, in0=pp[H2:HD, :], in1=c)
                nc.gpsimd.tensor_add(out=dst[H2:HD, :], in0=tc2, in1=td)

            def proj_chain_units(sj, m):
                """16 matmul micro-ops; rope/vt handling rides the last one."""
                state = {}

                def mk(dt):
                    def f():
                        if dt == 0:
                            state["pp"] = ps.tile(
                                [HD, SB], F32, tag="pp", bufs=2, name=f"pp_{sj}_{m}"
                            )
                        nc.tensor.matmul(
                            out=state["pp"],
                            lhsT=wqkv[:, dt, m * HD : (m + 1) * HD],
                            rhs=xq_ap(sj, dt),
                            start=(dt == 0),
                            stop=(dt == NT - 1),
                        )
                        if dt == NT - 1:
                            if m == 2:
                                vt = work.tile([HD, SB], BF16, tag="vt")
                                nc.scalar.copy(out=vt, in_=state["pp"])
                                vt_pending[sj] = vt
                            else:
                                rope_emit(state["pp"], sj, m)

                    return f

                return [("op", mk(dt)) for dt in range(NT)]

            def vtp_units(sj):
                """v[t, hd] transposes for AV's stationary (4 micro-ops)."""
                units = []
                for qq in range(SB // HD):
                    def f(qq=qq):
                        pt = ps.tile(
                            [HD, HD], BF16, tag="pp", bufs=2, name=f"pt_{sj}_{qq}"
                        )
                        nc.tensor.transpose(
                            pt, vt_pending[sj][:, qq * HD : (qq + 1) * HD], ident
                        )
                        nc.scalar.copy(out=v_sb[:, sj * 4 + qq, :], in_=pt)
                    units.append(("op", f))
                return units

            def enqueue_q_chains(sj):
                for h in range(1, NH):
                    proj_q.extend(proj_chain_units(sj, 2 + h))
                    proj_q.append(("marker", ("q", sj, h)))

            def enqueue_block_proj(sj):
                """Projection of block sj as micro-ops with readiness markers:
                ("tp", sj) = k/q0/v chains + transposes emitted (attention can
                start); ("q", sj, h) = head h's q chain + rope emitted."""
                proj_q.extend(proj_chain_units(sj, 0))       # k
                proj_q.extend(proj_chain_units(sj, 1))       # q0
                proj_q.extend(proj_chain_units(sj, 2))       # v
                proj_q.extend(vtp_units(sj))
                proj_q.append(("marker", ("tp", sj)))
                enqueue_q_chains(sj)

            def prolog_block0():
                """Block-0 k/q0/v chains interleaved at dt granularity so the
                PE consumes x/wqkv quarters as the startup DMAs land (the v
                chain borrows a PSUM bank from the idle "o" tag); q1-q3 ride
                the deferred queue, pulled in by attention(0)'s head-0 forces."""
                pps = {
                    0: ps.tile([HD, SB], F32, tag="pp", bufs=2, name="pp_0_0"),
                    1: ps.tile([HD, SB], F32, tag="pp", bufs=2, name="pp_0_1"),
                    2: ps.tile([HD, SB], F32, tag="o", bufs=2, name="pp_0_2"),
                }
                for dt in range(NT):
                    for m in (0, 1, 2):
                        nc.tensor.matmul(
                            out=pps[m],
                            lhsT=wqkv[:, dt, m * HD : (m + 1) * HD],
                            rhs=xq_ap(0, dt),
                            start=(dt == 0),
                            stop=(dt == NT - 1),
                        )
                rope_emit(pps[0], 0, 0)
                rope_emit(pps[1], 0, 1)
                vt = work.tile([HD, SB], BF16, tag="vt")
                nc.scalar.copy(out=vt, in_=pps[2])
                vt_pending[0] = vt
                for kind, f in vtp_units(0):
                    f()
                passed.add(("tp", 0))
                enqueue_q_chains(0)
                # the k/q0 ropes take ~8us of serial DVE/gpsimd after the
                # chains stop; run the q1/q2 chains meanwhile so the PE
                # doesn't idle between prolog and attention(0)
                drain_until(("q", 0, 1))
                drain_until(("q", 0, 2))

            def append_wo_block(sj):
                for stl in range(4):
                    st = sj * 4 + stl
                    t0 = st * HD
                    for dj in range(NSJ):
                        state = {}
                        for hh in range(NH):
                            def f(hh=hh, dj=dj, st=st, t0=t0, state=state):
                                if hh == 0:
                                    state["ps_y"] = ps.tile(
                                        [HD, SB], F32, tag="pp", bufs=2,
                                        name=f"ps_y_{st}_{dj}",
                                    )
                                nc.tensor.matmul(
                                    out=state["ps_y"],
                                    lhsT=on_sb[:, hh, t0 : t0 + HD],
                                    rhs=wo_sb[:, hh, dj * SB : (dj + 1) * SB],
                                    start=(hh == 0),
                                    stop=(hh == NH - 1),
                                )
                                if hh == NH - 1:
                                    y_sb = work.tile(
                                        [HD, SB], BF16, tag="ysb", bufs=4,
                                        name=f"ysb_{st}_{dj}",
                                    )
                                    # scalar takes 3 of 4 copies (DVE is the
                                    # rope/acc engine); all y DMA issues ride
                                    # the idle sync engine (scalar issue cost
                                    # ~650ns each would eat exp headroom)
                                    if dj % 4 == 3:
                                        nc.vector.tensor_copy(y_sb, state["ps_y"])
                                    else:
                                        nc.scalar.copy(out=y_sb, in_=state["ps_y"])
                                    nc.sync.dma_start(
                                        out=y_d[t0 : t0 + HD, dj * SB : (dj + 1) * SB],
                                        in_=y_sb,
                                    )
                            wo_q.append(f)

            # ---- main loop ----
            prolog_block0()

            STUFF_RATE = {0: 4, 1: 5, 2: 4, 3: 3}
            WO_KEEP = {0: 64, 1: 128, 2: 128, 3: 0}

            for sj in range(NSJ):
                s0 = sj * SB
                if sj + 1 < NSJ:
                    enqueue_block_proj(sj + 1)

                nt = 4 * sj + 4  # causal: t-tiles 0..nt-1
                LOOKAHEAD = 4
                r = STUFF_RATE[sj]
                deferred_norm = [None]
                hstate = {}

                def emit_front(h, ti, hstate=hstate, sj=sj):
                    qts_, acc_ = hstate[h]["q"], hstate[h]["acc"]
                    kdiag = ti - 4 * sj
                    c0 = max(0, kdiag) * HD  # first valid column (diag band)
                    ps_s = ps.tile(
                        [HD, SB], F32, tag="s", bufs=3, name=f"s_{sj}_{h}_{ti}"
                    )
                    nc.tensor.matmul(
                        out=ps_s[:, c0:SB],
                        lhsT=kt[:, ti * HD : (ti + 1) * HD],
                        rhs=qts_[:, c0:SB],
                        start=True,
                        stop=True,
                    )
                    es = es_pool.tile(
                        [HD, SB], BF16, tag="es", bufs=8, name=f"es_{sj}_{h}_{ti}"
                    )
                    nc.scalar.activation(
                        out=es[:, c0:SB], in_=ps_s[:, c0:SB], func=Exp,
                        scale=float(SCALE),
                    )
                    if kdiag >= 0:
                        # triangular part: first HD valid columns; block 0 is
                        # rope-congested on gpsimd, so alternate with DVE there
                        eng = nc.vector if (sj == 0 and ti % 2 == 1) else nc.gpsimd
                        eng.tensor_mul(
                            out=es[:, c0 : c0 + HD],
                            in0=es[:, c0 : c0 + HD],
                            in1=tri,
                        )
                    if ti == 0:
                        hstate[h]["es0"] = es  # acc init fused into ti=1's add
                    elif ti == 1:
                        es0 = hstate[h]["es0"]
                        nc.vector.tensor_add(
                            out=acc_[:, c0:SB], in0=es0[:, c0:SB],
                            in1=es[:, c0:SB],
                        )
                        if c0 > 0:
                            nc.vector.tensor_copy(acc_[:, 0:c0], es0[:, 0:c0])
                    else:
                        nc.vector.tensor_add(
                            out=acc_[:, c0:SB], in0=acc_[:, c0:SB],
                            in1=es[:, c0:SB],
                        )
                    return (h, ti, es, c0)

                def emit_back(item, hstate=hstate, nt=nt):
                    h, ti, es, c0 = item
                    nc.tensor.matmul(
                        out=hstate[h]["o"][:, c0:SB],
                        lhsT=v_sb[:, ti, :],
                        rhs=es[:, c0:SB],
                        start=(ti == 0),
                        stop=(ti == nt - 1),
                    )

                def make_norm(h, hstate=hstate, sj=sj, s0=s0):
                    def norm_emit():
                        # den = colsum(acc), broadcast via all-ones stationary
                        ps_den = ps.tile(
                            [HD, SB], F32, tag="den", bufs=1, name=f"den_{sj}_{h}"
                        )
                        nc.tensor.matmul(
                            out=ps_den, lhsT=ones_sb, rhs=hstate[h]["acc"],
                            start=True, stop=True,
                        )
                        rb = work.tile([HD, SB], F32, tag="rb")
                        nc.vector.reciprocal_approx_fast(out=rb, in_=ps_den)
                        nc.vector.tensor_mul(
                            out=on_sb[:, h, s0 : s0 + SB], in0=hstate[h]["o"],
                            in1=rb,
                        )
                    return norm_emit

                # flat (h, ti) pipeline: the back stream lags LOOKAHEAD tiles
                # and crosses head boundaries, so head starts have no bubble
                pend = []
                drain_until(("tp", sj))
                for h in range(NH):
                    hstate[h] = {
                        "q": q_tiles[(sj, h)],
                        "o": ps.tile([HD, SB], F32, tag="o", bufs=2,
                                     name=f"o_{sj}_{h}"),
                        "acc": es_pool.tile([HD, SB], BF16, tag="acc", bufs=2,
                                            name=f"acc_{sj}_{h}"),
                    }
                    for ti in range(nt):
                        pend.append(emit_front(h, ti))
                        if len(pend) > LOOKAHEAD:
                            emit_back(pend.pop(0))
                        if ti == 3 and deferred_norm[0] is not None:
                            deferred_norm[0]()
                            deferred_norm[0] = None
                        for _ in range(r):
                            # the last block reserves wo units to bridge the
                            # final norm's latency (a PE idle there drops the
                            # clock p-state and slows the whole wo tail)
                            pop_one(wo_floor=12 if sj == NSJ - 1 else 0)
                        # pull the q chains through early: all three pop
                        # during head 0 (PE-dense clusters; their ropes
                        # pipeline on DVE one head ahead of consumption)
                        if h == 0 and ti in (0, 1, 2):
                            drain_until(("q", sj, ti + 1))
                        elif h >= 1 and h + 1 < NH and ti == 0:
                            drain_until(("q", sj, h + 1))
                    deferred_norm[0] = make_norm(h)
                while pend:
                    emit_back(pend.pop(0))

                # cover the last head's colsum latency with a few queue pops
                for _ in range(8):
                    pop_one(wo_floor=4 if sj == NSJ - 1 else 0)
                deferred_norm[0]()
                deferred_norm[0] = None

                append_wo_block(sj)
                # keep wo backlog to stuff later attention blocks; block 3's
                # own chains are the only tail
                while len(wo_q) > WO_KEEP[sj]:
                    wo_q.pop(0)()
            while pop_one():
                pass

    nc.compile()
    return nc


def _get_program():
    global _PROG
    if _PROG is None:
        _PROG = _build_program()
    return _PROG


def _make_in_maps(x, freqs_cos, freqs_sin, wq, wk, wv, wo):
    perm = np.concatenate([np.arange(0, HD, 2), np.arange(1, HD, 2)])  # even|odd

    costT = np.ascontiguousarray(np.asarray(freqs_cos, np.float32).T).astype(F16)
    sintT = np.ascontiguousarray(np.asarray(freqs_sin, np.float32).T).astype(F16)

    tt = np.arange(HD)[:, None]
    ss = np.arange(HD)[None, :]
    tri = (tt <= ss).astype(BF)  # lower-tri in [t, s]: valid iff t <= s
    ident = np.eye(HD, dtype=BF)
    ones = np.ones((HD, HD), dtype=BF)

    # permute q/k head-dim columns so rope pairs land on partition halves
    def permute_heads(w, n_heads):
        w = np.asarray(w, np.float32).reshape(D, n_heads, HD)
        return w[:, :, perm].reshape(D, n_heads * HD)

    wq_p = permute_heads(wq, N_HEADS)
    wk_p = permute_heads(wk, N_KV_HEADS)
    wv_ = np.asarray(wv, np.float32)
    wo_ = np.asarray(wo, np.float32)
    x_ = np.asarray(x, np.float32)

    in_maps = []
    for c in range(8):
        b, g = divmod(c, 4)
        # column order [k, q0, v, q1, q2, q3]: the first 384-col half feeds
        # the interleaved block-0 prolog; q heads then arrive in use order
        wq_g = wq_p[:, g * NH * HD : (g + 1) * NH * HD]
        wqkv = np.concatenate(
            [
                wk_p[:, g * HD : (g + 1) * HD],
                wq_g[:, 0:HD],
                wv_[:, g * HD : (g + 1) * HD],
                wq_g[:, HD:],
            ],
            axis=1,
        )
        in_maps.append(
            {
                "xt": np.ascontiguousarray(x_[b].T).astype(BF),
                "wqkv": np.ascontiguousarray(wqkv).astype(BF),
                "wo": np.ascontiguousarray(
                    wo_[g * NH * HD : (g + 1) * NH * HD, :]
                ).astype(BF),
                "cost": costT,
                "sint": sintT,
                "tri": tri,
                "ident": ident,
                "ones": ones,
            }
        )
    return in_maps


def run(x, freqs_cos, freqs_sin, wq, wk, wv, wo, trace=False):
    from concourse.bass_utils import run_bass_kernel_spmd

    nc = _get_program()
    in_maps = _make_in_maps(x, freqs_cos, freqs_sin, wq, wk, wv, wo)
    res = run_bass_kernel_spmd(nc, in_maps, list(range(8)), trace=trace)
    out = np.empty((B, S, D), dtype=np.float32)
    for b in range(B):
        acc = res.results[b * 4]["y"].astype(np.float32)
        for g in range(1, 4):
            acc = acc + res.results[b * 4 + g]["y"].astype(np.float32)
        out[b] = acc
    return out, res


def kernel(x, freqs_cos, freqs_sin, wq, wk, wv, wo):
    out, _ = run(x, freqs_cos, freqs_sin, wq, wk, wv, wo, trace=False)
    return out


# revision 57
# speedup vs baseline: 1.0005x; 1.0005x over previous
"""GQA attention forward (B=2, S=2048, D=2048, 16 q heads / 4 kv heads, RoPE,
causal) on 8 Trainium2 NeuronCores.

Sharding: core c <-> (batch b = c//4, kv-group g = c%4). Each core computes its
4 query heads + 1 kv head end-to-end, including its row-shard of wo; the host
sums the 4 wo-partials per batch (the "all-reduce after wo" of the tensor
parallel scheme, done at gather time).

Layout tricks:
  - x is passed transposed (d-major) so every matmul contraction dim lands on
    SBUF partitions.
  - wq/wk columns are permuted per head (even dims -> partitions 0..63, odd ->
    64..127) so RoPE becomes plain elementwise DVE math on partition halves.
    The permutation cancels in q.k dot products.
  - all matmuls run in bf16 (fp8 DoubleRow measured exactly 2x on HW, so
    error-compensated fp8 (3 matmuls per 2 bf16-equivalents) is a net loss);
    accumulation stays fp32 in PSUM.
  - scores are built transposed ([t, s]); the softmax denominator is an
    all-ones-matrix matmul accumulated in PSUM, which lands the denominator
    already broadcast across partitions.
  - deferred-work queues (high-prio: next-block projection chains + v
    transposes; low-prio: wo chains) hold per-matmul micro-ops; the attention
    tile loop stuffs them into the PE slack left by the scalar-engine exp
    pacing (~220ns/tile). Each block's q chains are force-drained during the
    previous attention phase / head 0 so their rope latency (~4us serial
    DVE+gpsimd per chain) hides behind a full head of attention work.
    wo backlog is retained so block 3's large attention phase has stuff work,
    and a small wo reserve bridges the final norm's latency (a PE idle there
    drops the clock p-state and slows the whole wo drain tail ~630ns/matmul).
  - startup: the PE warmup (clock-gate ramp) runs on a memset tile (no DMA
    dependency, first matmul ~7.5us); block-0 x rides sync while wqkv h1 +
    fp16 cos/sin ride scalar, ordered by first use, so the dt-interleaved
    block-0 k/q0/v prolog starts on the first quarter (~13us) and streams at
    DMA arrival pace. x s-blocks 1-3 load s-block-major so block sj+1's
    projections never wait on a later quarter. y writes DMA via the
    otherwise-idle sync engine (scalar issue cost would eat exp headroom).

Measured (8 cores, core-0 profile): 262.6-263.6us; PE busy ~221us of that.
Dead ends measured on HW: fp8 DoubleRow is exactly 2x bf16 per matmul, so
error-compensated fp8 (3 matmuls per 2 bf16-equivalents) is a 1.5x net loss;
plain fp8 fails the 2e-2 gate (5.7e-2); half-width (256) attention segments
double per-op overheads and flip block 3 scalar-bound; AV LOOKAHEAD=5
corrupts numerics (es-pool lifetime); gpsimd cannot access PSUM.
"""

import ml_dtypes
import numpy as np

BF = ml_dtypes.bfloat16
F16 = np.float16
B, S, D = 2, 2048, 2048
N_HEADS, N_KV_HEADS, HD = 16, 4, 128
NH = N_HEADS // N_KV_HEADS  # q heads per core = 4
SB = 512                    # s-block (moving dim per matmul)
NSJ = S // SB               # 4 s-blocks
NT = S // HD                # 16 t-tiles (and d-tiles)
NM = NH + 2                 # 6 projection column-blocks: k, v, q0..q3
H2 = HD // 2
SCALE = 1.0 / np.sqrt(HD).astype(np.float32)

_PROG = None  # built once per process


def _build_program():
    import concourse.bacc as bacc
    import concourse.tile as tile
    from concourse import mybir

    F32 = mybir.dt.float32
    BF16 = mybir.dt.bfloat16
    FP16 = mybir.dt.float16
    Exp = mybir.ActivationFunctionType.Exp

    nc = bacc.Bacc("TRN2", target_bir_lowering=False, debug=False)

    xt_d = nc.declare_dram_parameter("xt", [D, S], BF16, isOutput=False)
    wqkv_d = nc.declare_dram_parameter("wqkv", [D, NM * HD], BF16, isOutput=False)
    wo_d = nc.declare_dram_parameter("wo", [NH * HD, D], BF16, isOutput=False)
    cost_d = nc.declare_dram_parameter("cost", [H2, S], FP16, isOutput=False)
    sint_d = nc.declare_dram_parameter("sint", [H2, S], FP16, isOutput=False)
    tri_d = nc.declare_dram_parameter("tri", [HD, HD], BF16, isOutput=False)
    ident_d = nc.declare_dram_parameter("ident", [HD, HD], BF16, isOutput=False)
    ones_d = nc.declare_dram_parameter("ones", [HD, HD], BF16, isOutput=False)
    y_d = nc.declare_dram_parameter("y", [S, D], BF16, isOutput=True)

    with tile.TileContext(nc) as tc:
        with (
            tc.tile_pool(name="consts", bufs=1) as consts,
            tc.tile_pool(name="persist", bufs=1) as persist,
            tc.tile_pool(name="work", bufs=2) as work,
            tc.tile_pool(name="xts_pool", bufs=1) as xts_pool,
            tc.tile_pool(name="qk_pool", bufs=1) as qk_pool,
            tc.tile_pool(name="es_pool", bufs=1) as es_pool,
            tc.tile_pool(name="ps", bufs=1, space="PSUM") as ps,
        ):
            tri = consts.tile([HD, HD], BF16, tag="tri")
            ident = consts.tile([HD, HD], BF16, tag="ident")
            ones_sb = consts.tile([HD, HD], BF16, tag="ones")
            cost = consts.tile([H2, S], FP16, tag="cost")
            sint = consts.tile([H2, S], FP16, tag="sint")

            wqkv = persist.tile([HD, NT, NM * HD], BF16, tag="wqkv")
            kt = persist.tile([HD, S], BF16, tag="kt")
            v_sb = persist.tile([HD, NT, HD], BF16, tag="v_sb")
            on_sb = persist.tile([HD, NH, S], BF16, tag="on")
            wo_sb = persist.tile([HD, NH, D], BF16, tag="wo")

            xt_r = xt_d[:, :].rearrange("(t p) s -> p t s", p=HD)
            wqkv_r = wqkv_d[:, :].rearrange("(t p) m -> p t m", p=HD)

            # ---- PE warm-up on a memset tile: no DMA dependency, so the
            # clock-gate (HAM) ramp starts as soon as the preamble ends ----
            dmy = consts.tile([HD, SB], BF16, tag="dmy")
            nc.vector.memset(dmy, 0.0)
            ps_warm = ps.tile([HD, SB], F32, tag="s", bufs=3, name="warmup")
            NWARM = 32
            for w in range(NWARM):
                nc.tensor.matmul(
                    out=ps_warm, lhsT=dmy[:, 0:HD], rhs=dmy,
                    start=(w == 0), stop=(w == NWARM - 1),
                )

            # ---- startup DMAs (hwdge queues: sync + scalar; gpsimd swdge
            # only for tiny consts). Ordered by first use so the interleaved
            # block-0 projection prolog can start after the first quarter;
            # two h2 quarters ride sync so all q-head weights land by ~23us ----
            MH = 3 * HD  # first column-half: k, q0, v
            xts_tiles = {}
            xrest_tiles = {}
            # sync: x block-0 quarters in dt order
            for ck in range(4):
                xq = xts_pool.tile(
                    [HD, NT // 4, SB], BF16, tag="xts", bufs=4, name=f"xts_0_{ck}"
                )
                nc.sync.dma_start(out=xq, in_=xt_r[:, ck * 4 : (ck + 1) * 4, 0:SB])
                xts_tiles[(0, ck)] = xq
            # scalar: first wqkv quarter (the PE's first real work), then
            # cos/sin (fp16, needed by rope-k ~20us), then the rest of h1
            nc.scalar.dma_start(
                out=wqkv[:, 0:4, 0:MH], in_=wqkv_r[:, 0:4, 0:MH]
            )
            nc.scalar.dma_start(out=cost, in_=cost_d[:, :])
            nc.scalar.dma_start(out=sint, in_=sint_d[:, :])
            for ck in range(1, 4):
                nc.scalar.dma_start(
                    out=wqkv[:, ck * 4 : (ck + 1) * 4, 0:MH],
                    in_=wqkv_r[:, ck * 4 : (ck + 1) * 4, 0:MH],
                )
            # preload the exp activation table while DMAs stream
            actwarm = work.tile([HD, 1], BF16, tag="actwarm", bufs=1)
            nc.scalar.activation(
                out=actwarm, in_=dmy[:, 0:1],
                func=mybir.ActivationFunctionType.Exp,
            )
            # wqkv second halves (q1..q3 columns): split scalar/sync
            for ck, eng in [(0, nc.scalar), (1, nc.scalar), (2, nc.sync), (3, nc.sync)]:
                eng.dma_start(
                    out=wqkv[:, ck * 4 : (ck + 1) * 4, MH : NM * HD],
                    in_=wqkv_r[:, ck * 4 : (ck + 1) * 4, MH : NM * HD],
                )
            nc.gpsimd.dma_start(out=tri, in_=tri_d[:, :])
            nc.gpsimd.dma_start(out=ident, in_=ident_d[:, :])
            nc.gpsimd.dma_start(out=ones_sb, in_=ones_d[:, :])
            # x s-blocks 1-3, s-block-major so earlier blocks land first
            for sj in range(1, NSJ):
                for ck in range(4):
                    xr = xts_pool.tile(
                        [HD, NT // 4, SB], BF16, tag="xrest", bufs=12,
                        name=f"xrest_{sj}_{ck}",
                    )
                    nc.sync.dma_start(
                        out=xr,
                        in_=xt_r[:, ck * 4 : (ck + 1) * 4, sj * SB : (sj + 1) * SB],
                    )
                    xrest_tiles[(sj, ck)] = xr
            nc.scalar.dma_start(
                out=wo_sb, in_=wo_d[:, :].rearrange("(h p) d -> p h d", p=HD)
            )

            def xq_ap(sj, dt):
                ck, sub = dt // 4, dt % 4
                if sj == 0:
                    return xts_tiles[(0, ck)][:, sub, :]
                return xrest_tiles[(sj, ck)][:, sub, :]

            # ---- deferred-work queues: proj (high prio) and wo (low) ----
            proj_q = []   # ('op', closure) | ('marker', key)
            wo_q = []     # closures
            passed = set()
            q_tiles = {}  # (sj, h) -> tile, filled lazily by rope closures
            vt_pending = {}

            def pop_proj():
                while proj_q:
                    kind, payload = proj_q.pop(0)
                    if kind == "marker":
                        passed.add(payload)
                        continue
                    payload()
                    return True
                return False

            def pop_one(wo_floor=0):
                if pop_proj():
                    return True
                if len(wo_q) > wo_floor:
                    wo_q.pop(0)()
                    return True
                return False

            def drain_until(marker):
                while marker not in passed and proj_q:
                    pop_proj()

            # wqkv column-block order (host-permuted to match consumption):
            # m=0: k, m=1: q0, m=2: v, m=3..5: q1..q3
            def m_to_qhead(m):
                return 0 if m == 1 else m - 2

            def rope_emit(pp, sj, m):
                # rows 0:64 = even dims (xr), 64:128 = odd (xi)
                # out_even = xr*c - xi*s ; out_odd = xr*s + xi*c
                # one PSUM->bf16 copy, then all muls run in DVE 2x mode
                # (fp32-PSUM-input ops cost 717ns vs 335ns for bf16 SBUF)
                s0 = sj * SB
                if m == 0:
                    dst = kt[:, s0 : s0 + SB]
                else:
                    h = m_to_qhead(m)
                    dst = qk_pool.tile(
                        [HD, SB], BF16, tag="qk", bufs=8, name=f"q_{sj}_{h}"
                    )
                    q_tiles[(sj, h)] = dst
                c = cost[:, s0 : s0 + SB]
                sn = sint[:, s0 : s0 + SB]
                ta = work.tile([H2, SB], F32, tag="ropeA")
                tb = work.tile([H2, SB], F32, tag="ropeB")
                nc.vector.tensor_mul(out=ta, in0=pp[0:H2, :], in1=c)
                nc.vector.tensor_mul(out=tb, in0=pp[H2:HD, :], in1=sn)
                nc.gpsimd.tensor_sub(out=dst[0:H2, :], in0=ta, in1=tb)
                tc2 = work.tile([H2, SB], F32, tag="ropeA")
                td = work.tile([H2, SB], F32, tag="ropeB")
                nc.vector.tensor_mul(out=tc2, in0=pp[0:H2, :], in1=sn)
                nc.vector.tensor_mul(out=td, in0=pp[H2:HD, :], in1=c)
                nc.gpsimd.tensor_add(out=dst[H2:HD, :], in0=tc2, in1=td)

            def proj_chain_units(sj, m):
                """16 matmul micro-ops; rope/vt handling rides the last one."""
                state = {}

                def mk(dt):
                    def f():
                        if dt == 0:
                            state["pp"] = ps.tile(
                                [HD, SB], F32, tag="pp", bufs=2, name=f"pp_{sj}_{m}"
                            )
                        nc.tensor.matmul(
                            out=state["pp"],
                            lhsT=wqkv[:, dt, m * HD : (m + 1) * HD],
                            rhs=xq_ap(sj, dt),
                            start=(dt == 0),
                            stop=(dt == NT - 1),
                        )
                        if dt == NT - 1:
                            if m == 2:
                                vt = work.tile([HD, SB], BF16, tag="vt")
                                nc.scalar.copy(out=vt, in_=state["pp"])
                                vt_pending[sj] = vt
                            else:
                                rope_emit(state["pp"], sj, m)

                    return f

                return [("op", mk(dt)) for dt in range(NT)]

            def vtp_units(sj):
                """v[t, hd] transposes for AV's stationary (4 micro-ops)."""
                units = []
                for qq in range(SB // HD):
                    def f(qq=qq):
                        pt = ps.tile(
                            [HD, HD], BF16, tag="pp", bufs=2, name=f"pt_{sj}_{qq}"
                        )
                        nc.tensor.transpose(
                            pt, vt_pending[sj][:, qq * HD : (qq + 1) * HD], ident
                        )
                        nc.scalar.copy(out=v_sb[:, sj * 4 + qq, :], in_=pt)
                    units.append(("op", f))
                return units

            def enqueue_q_chains(sj):
                for h in range(1, NH):
                    proj_q.extend(proj_chain_units(sj, 2 + h))
                    proj_q.append(("marker", ("q", sj, h)))

            def enqueue_block_proj(sj):
                """Projection of block sj as micro-ops with readiness markers:
                ("tp", sj) = k/q0/v chains + transposes emitted (attention can
                start); ("q", sj, h) = head h's q chain + rope emitted."""
                proj_q.extend(proj_chain_units(sj, 0))       # k
                proj_q.extend(proj_chain_units(sj, 1))       # q0
                proj_q.extend(proj_chain_units(sj, 2))       # v
                proj_q.extend(vtp_units(sj))
                proj_q.append(("marker", ("tp", sj)))
                enqueue_q_chains(sj)

            def prolog_block0():
                """Block-0 k/q0/v chains interleaved at dt granularity so the
                PE consumes x/wqkv quarters as the startup DMAs land (the v
                chain borrows a PSUM bank from the idle "o" tag); q1-q3 ride
                the deferred queue, pulled in by attention(0)'s head-0 forces."""
                pps = {
                    0: ps.tile([HD, SB], F32, tag="pp", bufs=2, name="pp_0_0"),
                    1: ps.tile([HD, SB], F32, tag="pp", bufs=2, name="pp_0_1"),
                    2: ps.tile([HD, SB], F32, tag="o", bufs=2, name="pp_0_2"),
                }
                for dt in range(NT):
                    for m in (0, 1, 2):
                        nc.tensor.matmul(
                            out=pps[m],
                            lhsT=wqkv[:, dt, m * HD : (m + 1) * HD],
                            rhs=xq_ap(0, dt),
                            start=(dt == 0),
                            stop=(dt == NT - 1),
                        )
                # q0's rope first (the first score needs it in full), then
                # rope-k split into two s-halves so kt tiles 0-1 are ready
                # ~4us earlier; attention(0) h0 starts on the first half
                rope_emit(pps[1], 0, 1)
                for lo, hi in ((0, SB // 2), (SB // 2, SB)):
                    wd = hi - lo
                    ta = work.tile([H2, SB], F32, tag="ropeA")
                    tb = work.tile([H2, SB], F32, tag="ropeB")
                    nc.vector.tensor_mul(
                        out=ta[:, 0:wd], in0=pps[0][0:H2, lo:hi], in1=cost[:, lo:hi]
                    )
                    nc.vector.tensor_mul(
                        out=tb[:, 0:wd], in0=pps[0][H2:HD, lo:hi], in1=sint[:, lo:hi]
                    )
                    nc.gpsimd.tensor_sub(
                        out=kt[0:H2, lo:hi], in0=ta[:, 0:wd], in1=tb[:, 0:wd]
                    )
                    tc2 = work.tile([H2, SB], F32, tag="ropeA")
                    td = work.tile([H2, SB], F32, tag="ropeB")
                    nc.vector.tensor_mul(
                        out=tc2[:, 0:wd], in0=pps[0][0:H2, lo:hi], in1=sint[:, lo:hi]
                    )
                    nc.vector.tensor_mul(
                        out=td[:, 0:wd], in0=pps[0][H2:HD, lo:hi], in1=cost[:, lo:hi]
                    )
                    nc.gpsimd.tensor_add(
                        out=kt[H2:HD, lo:hi], in0=tc2[:, 0:wd], in1=td[:, 0:wd]
                    )
                vt = work.tile([HD, SB], BF16, tag="vt")
                nc.scalar.copy(out=vt, in_=pps[2])
                vt_pending[0] = vt
                for kind, f in vtp_units(0):
                    f()
                passed.add(("tp", 0))
                enqueue_q_chains(0)
                # the k/q0 ropes take ~8us of serial DVE/gpsimd after the
                # chains stop; run the q1/q2 chains meanwhile so the PE
                # doesn't idle between prolog and attention(0)
                drain_until(("q", 0, 1))
                drain_until(("q", 0, 2))

            def append_wo_block(sj):
                for stl in range(4):
                    st = sj * 4 + stl
                    t0 = st * HD
                    for dj in range(NSJ):
                        state = {}
                        for hh in range(NH):
                            def f(hh=hh, dj=dj, st=st, t0=t0, state=state):
                                if hh == 0:
                                    state["ps_y"] = ps.tile(
                                        [HD, SB], F32, tag="pp", bufs=2,
                                        name=f"ps_y_{st}_{dj}",
                                    )
                                nc.tensor.matmul(
                                    out=state["ps_y"],
                                    lhsT=on_sb[:, hh, t0 : t0 + HD],
                                    rhs=wo_sb[:, hh, dj * SB : (dj + 1) * SB],
                                    start=(hh == 0),
                                    stop=(hh == NH - 1),
                                )
                                if hh == NH - 1:
                                    y_sb = work.tile(
                                        [HD, SB], BF16, tag="ysb", bufs=4,
                                        name=f"ysb_{st}_{dj}",
                                    )
                                    # scalar takes 3 of 4 copies (DVE is the
                                    # rope/acc engine); all y DMA issues ride
                                    # the idle sync engine (scalar issue cost
                                    # ~650ns each would eat exp headroom)
                                    if dj % 4 == 3:
                                        nc.vector.tensor_copy(y_sb, state["ps_y"])
                                    else:
                                        nc.scalar.copy(out=y_sb, in_=state["ps_y"])
                                    nc.sync.dma_start(
                                        out=y_d[t0 : t0 + HD, dj * SB : (dj + 1) * SB],
                                        in_=y_sb,
                                    )
                            wo_q.append(f)

            # ---- main loop ----
            prolog_block0()

            STUFF_RATE = {0: 4, 1: 5, 2: 4, 3: 3}
            WO_KEEP = {0: 64, 1: 128, 2: 128, 3: 0}

            for sj in range(NSJ):
                s0 = sj * SB
                if sj + 1 < NSJ:
                    enqueue_block_proj(sj + 1)

                nt = 4 * sj + 4  # causal: t-tiles 0..nt-1
                LOOKAHEAD = 4
                r = STUFF_RATE[sj]
                deferred_norm = [None]
                hstate = {}

                def emit_front(h, ti, hstate=hstate, sj=sj):
                    qts_, acc_ = hstate[h]["q"], hstate[h]["acc"]
                    kdiag = ti - 4 * sj
                    c0 = max(0, kdiag) * HD  # first valid column (diag band)
                    ps_s = ps.tile(
                        [HD, SB], F32, tag="s", bufs=3, name=f"s_{sj}_{h}_{ti}"
                    )
                    nc.tensor.matmul(
                        out=ps_s[:, c0:SB],
                        lhsT=kt[:, ti * HD : (ti + 1) * HD],
                        rhs=qts_[:, c0:SB],
                        start=True,
                        stop=True,
                    )
                    es = es_pool.tile(
                        [HD, SB], BF16, tag="es", bufs=8, name=f"es_{sj}_{h}_{ti}"
                    )
                    nc.scalar.activation(
                        out=es[:, c0:SB], in_=ps_s[:, c0:SB], func=Exp,
                        scale=float(SCALE),
                    )
                    if kdiag >= 0:
                        # triangular part: first HD valid columns; block 0 is
                        # rope-congested on gpsimd, so alternate with DVE there
                        eng = nc.vector if (sj == 0 and ti % 2 == 1) else nc.gpsimd
                        eng.tensor_mul(
                            out=es[:, c0 : c0 + HD],
                            in0=es[:, c0 : c0 + HD],
                            in1=tri,
                        )
                    if ti == 0:
                        hstate[h]["es0"] = es  # acc init fused into ti=1's add
                    elif ti == 1:
                        es0 = hstate[h]["es0"]
                        nc.vector.tensor_add(
                            out=acc_[:, c0:SB], in0=es0[:, c0:SB],
                            in1=es[:, c0:SB],
                        )
                        if c0 > 0:
                            nc.vector.tensor_copy(acc_[:, 0:c0], es0[:, 0:c0])
                    else:
                        nc.vector.tensor_add(
                            out=acc_[:, c0:SB], in0=acc_[:, c0:SB],
                            in1=es[:, c0:SB],
                        )
                    return (h, ti, es, c0)

                def emit_back(item, hstate=hstate, nt=nt):
                    h, ti, es, c0 = item
                    nc.tensor.matmul(
                        out=hstate[h]["o"][:, c0:SB],
                        lhsT=v_sb[:, ti, :],
                        rhs=es[:, c0:SB],
                        start=(ti == 0),
                        stop=(ti == nt - 1),
                    )

                def make_norm(h, hstate=hstate, sj=sj, s0=s0):
                    def norm_emit():
                        # den = colsum(acc), broadcast via all-ones stationary
                        ps_den = ps.tile(
                            [HD, SB], F32, tag="den", bufs=1, name=f"den_{sj}_{h}"
                        )
                        nc.tensor.matmul(
                            out=ps_den, lhsT=ones_sb, rhs=hstate[h]["acc"],
                            start=True, stop=True,
                        )
                        rb = work.tile([HD, SB], F32, tag="rb")
                        nc.vector.reciprocal_approx_fast(out=rb, in_=ps_den)
                        nc.vector.tensor_mul(
                            out=on_sb[:, h, s0 : s0 + SB], in0=hstate[h]["o"],
                            in1=rb,
                        )
                    return norm_emit

                # flat (h, ti) pipeline: the back stream lags LOOKAHEAD tiles
                # and crosses head boundaries, so head starts have no bubble
                pend = []
                drain_until(("tp", sj))
                for h in range(NH):
                    hstate[h] = {
                        "q": q_tiles[(sj, h)],
                        "o": ps.tile([HD, SB], F32, tag="o", bufs=2,
                                     name=f"o_{sj}_{h}"),
                        "acc": es_pool.tile([HD, SB], BF16, tag="acc", bufs=2,
                                            name=f"acc_{sj}_{h}"),
                    }
                    for ti in range(nt):
                        pend.append(emit_front(h, ti))
                        if len(pend) > LOOKAHEAD:
                            emit_back(pend.pop(0))
                        if ti == 3 and deferred_norm[0] is not None:
                            deferred_norm[0]()
                            deferred_norm[0] = None
                        for _ in range(r):
                            # the last block reserves wo units to bridge the
                            # final norm's latency (a PE idle there drops the
                            # clock p-state and slows the whole wo tail)
                            pop_one(wo_floor=12 if sj == NSJ - 1 else 0)
                        # pull the q chains through early: all three pop
                        # during head 0 (PE-dense clusters; their ropes
                        # pipeline on DVE one head ahead of consumption)
                        if h == 0 and ti in (0, 1, 2):
                            drain_until(("q", sj, ti + 1))
                        elif h >= 1 and h + 1 < NH and ti == 0:
                            drain_until(("q", sj, h + 1))
                    deferred_norm[0] = make_norm(h)
                while pend:
                    emit_back(pend.pop(0))

                # cover the last head's colsum latency with a few queue pops
                for _ in range(8):
                    pop_one(wo_floor=4 if sj == NSJ - 1 else 0)
                deferred_norm[0]()
                deferred_norm[0] = None

                append_wo_block(sj)
                # keep wo backlog to stuff later attention blocks; block 3's
                # own chains are the only tail
                while len(wo_q) > WO_KEEP[sj]:
                    wo_q.pop(0)()
            while pop_one():
                pass

    nc.compile()
    return nc


def _get_program():
    global _PROG
    if _PROG is None:
        _PROG = _build_program()
    return _PROG


def _make_in_maps(x, freqs_cos, freqs_sin, wq, wk, wv, wo):
    perm = np.concatenate([np.arange(0, HD, 2), np.arange(1, HD, 2)])  # even|odd

    costT = np.ascontiguousarray(np.asarray(freqs_cos, np.float32).T).astype(F16)
    sintT = np.ascontiguousarray(np.asarray(freqs_sin, np.float32).T).astype(F16)

    tt = np.arange(HD)[:, None]
    ss = np.arange(HD)[None, :]
    tri = (tt <= ss).astype(BF)  # lower-tri in [t, s]: valid iff t <= s
    ident = np.eye(HD, dtype=BF)
    ones = np.ones((HD, HD), dtype=BF)

    # permute q/k head-dim columns so rope pairs land on partition halves
    def permute_heads(w, n_heads):
        w = np.asarray(w, np.float32).reshape(D, n_heads, HD)
        return w[:, :, perm].reshape(D, n_heads * HD)

    wq_p = permute_heads(wq, N_HEADS)
    wk_p = permute_heads(wk, N_KV_HEADS)
    wv_ = np.asarray(wv, np.float32)
    wo_ = np.asarray(wo, np.float32)
    x_ = np.asarray(x, np.float32)

    in_maps = []
    for c in range(8):
        b, g = divmod(c, 4)
        # column order [k, q0, v, q1, q2, q3]: the first 384-col half feeds
        # the interleaved block-0 prolog; q heads then arrive in use order
        wq_g = wq_p[:, g * NH * HD : (g + 1) * NH * HD]
        wqkv = np.concatenate(
            [
                wk_p[:, g * HD : (g + 1) * HD],
                wq_g[:, 0:HD],
                wv_[:, g * HD : (g + 1) * HD],
                wq_g[:, HD:],
            ],
            axis=1,
        )
        in_maps.append(
            {
                "xt": np.ascontiguousarray(x_[b].T).astype(BF),
                "wqkv": np.ascontiguousarray(wqkv).astype(BF),
                "wo": np.ascontiguousarray(
                    wo_[g * NH * HD : (g + 1) * NH * HD, :]
                ).astype(BF),
                "cost": costT,
                "sint": sintT,
                "tri": tri,
                "ident": ident,
                "ones": ones,
            }
        )
    return in_maps


def run(x, freqs_cos, freqs_sin, wq, wk, wv, wo, trace=False):
    from concourse.bass_utils import run_bass_kernel_spmd

    nc = _get_program()
    in_maps = _make_in_maps(x, freqs_cos, freqs_sin, wq, wk, wv, wo)
    res = run_bass_kernel_spmd(nc, in_maps, list(range(8)), trace=trace)
    out = np.empty((B, S, D), dtype=np.float32)
    for b in range(B):
        acc = res.results[b * 4]["y"].astype(np.float32)
        for g in range(1, 4):
            acc = acc + res.results[b * 4 + g]["y"].astype(np.float32)
        out[b] = acc
    return out, res


def kernel(x, freqs_cos, freqs_sin, wq, wk, wv, wo):
    out, _ = run(x, freqs_cos, freqs_sin, wq, wk, wv, wo, trace=False)
    return out


# revision 60
# speedup vs baseline: 1.0019x; 1.0014x over previous
"""GQA attention forward (B=2, S=2048, D=2048, 16 q heads / 4 kv heads, RoPE,
causal) on 8 Trainium2 NeuronCores.

Sharding: core c <-> (batch b = c//4, kv-group g = c%4). Each core computes its
4 query heads + 1 kv head end-to-end, including its row-shard of wo; the host
sums the 4 wo-partials per batch (the "all-reduce after wo" of the tensor
parallel scheme, done at gather time).

Layout tricks:
  - x is passed transposed (d-major) so every matmul contraction dim lands on
    SBUF partitions.
  - wq/wk columns are permuted per head (even dims -> partitions 0..63, odd ->
    64..127) so RoPE becomes plain elementwise DVE math on partition halves.
    The permutation cancels in q.k dot products.
  - all matmuls run in bf16 (fp8 DoubleRow measured exactly 2x on HW, so
    error-compensated fp8 (3 matmuls per 2 bf16-equivalents) is a net loss);
    accumulation stays fp32 in PSUM.
  - scores are built transposed ([t, s]); the softmax denominator is an
    all-ones-matrix matmul accumulated in PSUM, which lands the denominator
    already broadcast across partitions.
  - deferred-work queues (high-prio: next-block projection chains + v
    transposes; low-prio: wo chains) hold per-matmul micro-ops; the attention
    tile loop stuffs them into the PE slack left by the scalar-engine exp
    pacing (~220ns/tile). Each block's q chains are force-drained during the
    previous attention phase / head 0 so their rope latency (~4us serial
    DVE+gpsimd per chain) hides behind a full head of attention work.
    wo backlog is retained so block 3's large attention phase has stuff work,
    and a small wo reserve bridges the final norm's latency (a PE idle there
    drops the clock p-state and slows the whole wo drain tail ~630ns/matmul).
  - startup: the PE warmup (clock-gate ramp) runs on a memset tile (no DMA
    dependency, first matmul ~7.5us); block-0 x rides sync while wqkv h1 +
    fp16 cos/sin ride scalar, ordered by first use, so the dt-interleaved
    block-0 k/q0/v prolog starts on the first quarter (~13us) and streams at
    DMA arrival pace. x s-blocks 1-3 load s-block-major so block sj+1's
    projections never wait on a later quarter. y writes DMA via the
    otherwise-idle sync engine (scalar issue cost would eat exp headroom).

Measured (8 cores, core-0 profile): 262.6-263.6us; PE busy ~221us of that.
Dead ends measured on HW: fp8 DoubleRow is exactly 2x bf16 per matmul, so
error-compensated fp8 (3 matmuls per 2 bf16-equivalents) is a 1.5x net loss;
plain fp8 fails the 2e-2 gate (5.7e-2); half-width (256) attention segments
double per-op overheads and flip block 3 scalar-bound; AV LOOKAHEAD=5
corrupts numerics (es-pool lifetime); gpsimd cannot access PSUM.
"""

import ml_dtypes
import numpy as np

BF = ml_dtypes.bfloat16
F16 = np.float16
B, S, D = 2, 2048, 2048
N_HEADS, N_KV_HEADS, HD = 16, 4, 128
NH = N_HEADS // N_KV_HEADS  # q heads per core = 4
SB = 512                    # s-block (moving dim per matmul)
NSJ = S // SB               # 4 s-blocks
NT = S // HD                # 16 t-tiles (and d-tiles)
NM = NH + 2                 # 6 projection column-blocks: k, v, q0..q3
H2 = HD // 2
SCALE = 1.0 / np.sqrt(HD).astype(np.float32)

_PROG = None  # built once per process


def _build_program():
    import concourse.bacc as bacc
    import concourse.tile as tile
    from concourse import mybir

    F32 = mybir.dt.float32
    BF16 = mybir.dt.bfloat16
    FP16 = mybir.dt.float16
    Exp = mybir.ActivationFunctionType.Exp

    nc = bacc.Bacc("TRN2", target_bir_lowering=False, debug=False)

    xt_d = nc.declare_dram_parameter("xt", [D, S], BF16, isOutput=False)
    wqkv_d = nc.declare_dram_parameter("wqkv", [D, NM * HD], BF16, isOutput=False)
    wo_d = nc.declare_dram_parameter("wo", [NH * HD, D], BF16, isOutput=False)
    cost_d = nc.declare_dram_parameter("cost", [H2, S], FP16, isOutput=False)
    sint_d = nc.declare_dram_parameter("sint", [H2, S], FP16, isOutput=False)
    tri_d = nc.declare_dram_parameter("tri", [HD, HD], BF16, isOutput=False)
    ident_d = nc.declare_dram_parameter("ident", [HD, HD], BF16, isOutput=False)
    ones_d = nc.declare_dram_parameter("ones", [HD, HD], BF16, isOutput=False)
    y_d = nc.declare_dram_parameter("y", [S, D], BF16, isOutput=True)

    with tile.TileContext(nc) as tc:
        with (
            tc.tile_pool(name="consts", bufs=1) as consts,
            tc.tile_pool(name="persist", bufs=1) as persist,
            tc.tile_pool(name="work", bufs=2) as work,
            tc.tile_pool(name="xts_pool", bufs=1) as xts_pool,
            tc.tile_pool(name="qk_pool", bufs=1) as qk_pool,
            tc.tile_pool(name="es_pool", bufs=1) as es_pool,
            tc.tile_pool(name="ps", bufs=1, space="PSUM") as ps,
        ):
            tri = consts.tile([HD, HD], BF16, tag="tri")
            ident = consts.tile([HD, HD], BF16, tag="ident")
            ones_sb = consts.tile([HD, HD], BF16, tag="ones")
            cost = consts.tile([H2, S], FP16, tag="cost")
            sint = consts.tile([H2, S], FP16, tag="sint")

            wqkv = persist.tile([HD, NT, NM * HD], BF16, tag="wqkv")
            kt = persist.tile([HD, S], BF16, tag="kt")
            v_sb = persist.tile([HD, NT, HD], BF16, tag="v_sb")
            on_sb = persist.tile([HD, NH, S], BF16, tag="on")
            wo_sb = persist.tile([HD, NH, D], BF16, tag="wo")

            xt_r = xt_d[:, :].rearrange("(t p) s -> p t s", p=HD)
            wqkv_r = wqkv_d[:, :].rearrange("(t p) m -> p t m", p=HD)

            # ---- PE warm-up on a memset tile: no DMA dependency, so the
            # clock-gate (HAM) ramp starts as soon as the preamble ends ----
            dmy = consts.tile([HD, SB], BF16, tag="dmy")
            nc.vector.memset(dmy, 0.0)
            ps_warm = ps.tile([HD, SB], F32, tag="s", bufs=3, name="warmup")
            NWARM = 26
            for w in range(NWARM):
                nc.tensor.matmul(
                    out=ps_warm, lhsT=dmy[:, 0:HD], rhs=dmy,
                    start=(w == 0), stop=(w == NWARM - 1),
                )

            # ---- startup DMAs (hwdge queues: sync + scalar; gpsimd swdge
            # only for tiny consts). Ordered by first use so the interleaved
            # block-0 projection prolog can start after the first quarter;
            # two h2 quarters ride sync so all q-head weights land by ~23us ----
            MH = 3 * HD  # first column-half: k, q0, v
            xts_tiles = {}
            xrest_tiles = {}
            # sync: x block-0 quarters in dt order
            for ck in range(4):
                xq = xts_pool.tile(
                    [HD, NT // 4, SB], BF16, tag="xts", bufs=4, name=f"xts_0_{ck}"
                )
                nc.sync.dma_start(out=xq, in_=xt_r[:, ck * 4 : (ck + 1) * 4, 0:SB])
                xts_tiles[(0, ck)] = xq
            # scalar: first wqkv quarter (the PE's first real work), then
            # cos/sin (fp16, needed by rope-k ~20us), then the rest of h1
            nc.scalar.dma_start(
                out=wqkv[:, 0:4, 0:MH], in_=wqkv_r[:, 0:4, 0:MH]
            )
            nc.scalar.dma_start(out=cost, in_=cost_d[:, :])
            nc.scalar.dma_start(out=sint, in_=sint_d[:, :])
            for ck in range(1, 4):
                nc.scalar.dma_start(
                    out=wqkv[:, ck * 4 : (ck + 1) * 4, 0:MH],
                    in_=wqkv_r[:, ck * 4 : (ck + 1) * 4, 0:MH],
                )
            # preload the exp activation table while DMAs stream
            actwarm = work.tile([HD, 1], BF16, tag="actwarm", bufs=1)
            nc.scalar.activation(
                out=actwarm, in_=dmy[:, 0:1],
                func=mybir.ActivationFunctionType.Exp,
            )
            # wqkv second halves (q1..q3 columns): split scalar/sync
            for ck, eng in [(0, nc.scalar), (1, nc.scalar), (2, nc.sync), (3, nc.sync)]:
                eng.dma_start(
                    out=wqkv[:, ck * 4 : (ck + 1) * 4, MH : NM * HD],
                    in_=wqkv_r[:, ck * 4 : (ck + 1) * 4, MH : NM * HD],
                )
            nc.gpsimd.dma_start(out=tri, in_=tri_d[:, :])
            nc.gpsimd.dma_start(out=ident, in_=ident_d[:, :])
            nc.gpsimd.dma_start(out=ones_sb, in_=ones_d[:, :])
            # x s-blocks 1-3, s-block-major so earlier blocks land first
            for sj in range(1, NSJ):
                for ck in range(4):
                    xr = xts_pool.tile(
                        [HD, NT // 4, SB], BF16, tag="xrest", bufs=12,
                        name=f"xrest_{sj}_{ck}",
                    )
                    nc.sync.dma_start(
                        out=xr,
                        in_=xt_r[:, ck * 4 : (ck + 1) * 4, sj * SB : (sj + 1) * SB],
                    )
                    xrest_tiles[(sj, ck)] = xr
            nc.scalar.dma_start(
                out=wo_sb, in_=wo_d[:, :].rearrange("(h p) d -> p h d", p=HD)
            )

            def xq_ap(sj, dt):
                ck, sub = dt // 4, dt % 4
                if sj == 0:
                    return xts_tiles[(0, ck)][:, sub, :]
                return xrest_tiles[(sj, ck)][:, sub, :]

            # ---- deferred-work queues: proj (high prio) and wo (low) ----
            proj_q = []   # ('op', closure) | ('marker', key)
            wo_q = []     # closures
            passed = set()
            q_tiles = {}  # (sj, h) -> tile, filled lazily by rope closures
            vt_pending = {}

            def pop_proj():
                while proj_q:
                    kind, payload = proj_q.pop(0)
                    if kind == "marker":
                        passed.add(payload)
                        continue
                    payload()
                    return True
                return False

            def pop_one(wo_floor=0):
                if pop_proj():
                    return True
                if len(wo_q) > wo_floor:
                    wo_q.pop(0)()
                    return True
                return False

            def drain_until(marker):
                while marker not in passed and proj_q:
                    pop_proj()

            # wqkv column-block order (host-permuted to match consumption):
            # m=0: k, m=1: q0, m=2: v, m=3..5: q1..q3
            def m_to_qhead(m):
                return 0 if m == 1 else m - 2

            def rope_emit(pp, sj, m):
                # rows 0:64 = even dims (xr), 64:128 = odd (xi)
                # out_even = xr*c - xi*s ; out_odd = xr*s + xi*c
                # one PSUM->bf16 copy, then all muls run in DVE 2x mode
                # (fp32-PSUM-input ops cost 717ns vs 335ns for bf16 SBUF)
                s0 = sj * SB
                if m == 0:
                    dst = kt[:, s0 : s0 + SB]
                else:
                    h = m_to_qhead(m)
                    dst = qk_pool.tile(
                        [HD, SB], BF16, tag="qk", bufs=8, name=f"q_{sj}_{h}"
                    )
                    q_tiles[(sj, h)] = dst
                c = cost[:, s0 : s0 + SB]
                sn = sint[:, s0 : s0 + SB]
                ta = work.tile([H2, SB], F32, tag="ropeA")
                tb = work.tile([H2, SB], F32, tag="ropeB")
                nc.vector.tensor_mul(out=ta, in0=pp[0:H2, :], in1=c)
                nc.vector.tensor_mul(out=tb, in0=pp[H2:HD, :], in1=sn)
                nc.gpsimd.tensor_sub(out=dst[0:H2, :], in0=ta, in1=tb)
                tc2 = work.tile([H2, SB], F32, tag="ropeA")
                td = work.tile([H2, SB], F32, tag="ropeB")
                nc.vector.tensor_mul(out=tc2, in0=pp[0:H2, :], in1=sn)
                nc.vector.tensor_mul(out=td, in0=pp[H2:HD, :], in1=c)
                nc.gpsimd.tensor_add(out=dst[H2:HD, :], in0=tc2, in1=td)

            def proj_chain_units(sj, m):
                """16 matmul micro-ops; rope/vt handling rides the last one."""
                state = {}

                def mk(dt):
                    def f():
                        if dt == 0:
                            state["pp"] = ps.tile(
                                [HD, SB], F32, tag="pp", bufs=2, name=f"pp_{sj}_{m}"
                            )
                        nc.tensor.matmul(
                            out=state["pp"],
                            lhsT=wqkv[:, dt, m * HD : (m + 1) * HD],
                            rhs=xq_ap(sj, dt),
                            start=(dt == 0),
                            stop=(dt == NT - 1),
                        )
                        if dt == NT - 1:
                            if m == 2:
                                vt = work.tile([HD, SB], BF16, tag="vt")
                                nc.scalar.copy(out=vt, in_=state["pp"])
                                vt_pending[sj] = vt
                            else:
                                rope_emit(state["pp"], sj, m)

                    return f

                return [("op", mk(dt)) for dt in range(NT)]

            def vtp_units(sj):
                """v[t, hd] transposes for AV's stationary (4 micro-ops)."""
                units = []
                for qq in range(SB // HD):
                    def f(qq=qq):
                        pt = ps.tile(
                            [HD, HD], BF16, tag="pp", bufs=2, name=f"pt_{sj}_{qq}"
                        )
                        nc.tensor.transpose(
                            pt, vt_pending[sj][:, qq * HD : (qq + 1) * HD], ident
                        )
                        nc.scalar.copy(out=v_sb[:, sj * 4 + qq, :], in_=pt)
                    units.append(("op", f))
                return units

            def enqueue_q_chains(sj):
                for h in range(1, NH):
                    proj_q.extend(proj_chain_units(sj, 2 + h))
                    proj_q.append(("marker", ("q", sj, h)))

            def enqueue_block_proj(sj):
                """Projection of block sj as micro-ops with readiness markers:
                ("tp", sj) = k/q0/v chains + transposes emitted (attention can
                start); ("q", sj, h) = head h's q chain + rope emitted."""
                proj_q.extend(proj_chain_units(sj, 0))       # k
                proj_q.extend(proj_chain_units(sj, 1))       # q0
                proj_q.extend(proj_chain_units(sj, 2))       # v
                proj_q.extend(vtp_units(sj))
                proj_q.append(("marker", ("tp", sj)))
                enqueue_q_chains(sj)

            def prolog_block0():
                """Block-0 k/q0/v chains interleaved at dt granularity so the
                PE consumes x/wqkv quarters as the startup DMAs land (the v
                chain borrows a PSUM bank from the idle "o" tag); q1-q3 ride
                the deferred queue, pulled in by attention(0)'s head-0 forces."""
                pps = {
                    0: ps.tile([HD, SB], F32, tag="pp", bufs=2, name="pp_0_0"),
                    1: ps.tile([HD, SB], F32, tag="pp", bufs=2, name="pp_0_1"),
                    2: ps.tile([HD, SB], F32, tag="o", bufs=2, name="pp_0_2"),
                }
                # chains run SEQUENTIALLY (q0, then k, then v) so q0's chain
                # stops ~7us earlier than a dt-interleave would allow and its
                # rope (the first score's gate) starts immediately; the k/v
                # chain matmuls then overlap the rope work on DVE/gpsimd
                for m in (1, 0, 2):
                    for dt in range(NT):
                        nc.tensor.matmul(
                            out=pps[m],
                            lhsT=wqkv[:, dt, m * HD : (m + 1) * HD],
                            rhs=xq_ap(0, dt),
                            start=(dt == 0),
                            stop=(dt == NT - 1),
                        )
                    if m == 1:
                        rope_emit(pps[1], 0, 1)
                # rope-k in two s-halves so kt tiles 0-1 are ready earlier
                for lo, hi in ((0, SB // 2), (SB // 2, SB)):
                    wd = hi - lo
                    ta = work.tile([H2, SB], F32, tag="ropeA")
                    tb = work.tile([H2, SB], F32, tag="ropeB")
                    nc.vector.tensor_mul(
                        out=ta[:, 0:wd], in0=pps[0][0:H2, lo:hi], in1=cost[:, lo:hi]
                    )
                    nc.vector.tensor_mul(
                        out=tb[:, 0:wd], in0=pps[0][H2:HD, lo:hi], in1=sint[:, lo:hi]
                    )
                    nc.gpsimd.tensor_sub(
                        out=kt[0:H2, lo:hi], in0=ta[:, 0:wd], in1=tb[:, 0:wd]
                    )
                    tc2 = work.tile([H2, SB], F32, tag="ropeA")
                    td = work.tile([H2, SB], F32, tag="ropeB")
                    nc.vector.tensor_mul(
                        out=tc2[:, 0:wd], in0=pps[0][0:H2, lo:hi], in1=sint[:, lo:hi]
                    )
                    nc.vector.tensor_mul(
                        out=td[:, 0:wd], in0=pps[0][H2:HD, lo:hi], in1=cost[:, lo:hi]
                    )
                    nc.gpsimd.tensor_add(
                        out=kt[H2:HD, lo:hi], in0=tc2[:, 0:wd], in1=td[:, 0:wd]
                    )
                vt = work.tile([HD, SB], BF16, tag="vt")
                nc.scalar.copy(out=vt, in_=pps[2])
                vt_pending[0] = vt
                for kind, f in vtp_units(0):
                    f()
                passed.add(("tp", 0))
                enqueue_q_chains(0)
                # the k/q0 ropes take ~8us of serial DVE/gpsimd after the
                # chains stop; run the q1/q2 chains meanwhile so the PE
                # doesn't idle between prolog and attention(0)
                drain_until(("q", 0, 1))
                drain_until(("q", 0, 2))

            def append_wo_block(sj):
                for stl in range(4):
                    st = sj * 4 + stl
                    t0 = st * HD
                    for dj in range(NSJ):
                        state = {}
                        for hh in range(NH):
                            def f(hh=hh, dj=dj, st=st, t0=t0, state=state):
                                if hh == 0:
                                    state["ps_y"] = ps.tile(
                                        [HD, SB], F32, tag="pp", bufs=2,
                                        name=f"ps_y_{st}_{dj}",
                                    )
                                nc.tensor.matmul(
                                    out=state["ps_y"],
                                    lhsT=on_sb[:, hh, t0 : t0 + HD],
                                    rhs=wo_sb[:, hh, dj * SB : (dj + 1) * SB],
                                    start=(hh == 0),
                                    stop=(hh == NH - 1),
                                )
                                if hh == NH - 1:
                                    y_sb = work.tile(
                                        [HD, SB], BF16, tag="ysb", bufs=4,
                                        name=f"ysb_{st}_{dj}",
                                    )
                                    # scalar takes 3 of 4 copies (DVE is the
                                    # rope/acc engine); all y DMA issues ride
                                    # the idle sync engine (scalar issue cost
                                    # ~650ns each would eat exp headroom)
                                    if dj % 4 == 3:
                                        nc.vector.tensor_copy(y_sb, state["ps_y"])
                                    else:
                                        nc.scalar.copy(out=y_sb, in_=state["ps_y"])
                                    nc.sync.dma_start(
                                        out=y_d[t0 : t0 + HD, dj * SB : (dj + 1) * SB],
                                        in_=y_sb,
                                    )
                            wo_q.append(f)

            # ---- main loop ----
            prolog_block0()

            STUFF_RATE = {0: 4, 1: 5, 2: 4, 3: 3}
            WO_KEEP = {0: 64, 1: 128, 2: 128, 3: 0}

            for sj in range(NSJ):
                s0 = sj * SB
                if sj + 1 < NSJ:
                    enqueue_block_proj(sj + 1)

                nt = 4 * sj + 4  # causal: t-tiles 0..nt-1
                LOOKAHEAD = 4
                r = STUFF_RATE[sj]
                deferred_norm = [None]
                hstate = {}

                def emit_front(h, ti, hstate=hstate, sj=sj):
                    qts_, acc_ = hstate[h]["q"], hstate[h]["acc"]
                    kdiag = ti - 4 * sj
                    c0 = max(0, kdiag) * HD  # first valid column (diag band)
                    ps_s = ps.tile(
                        [HD, SB], F32, tag="s", bufs=3, name=f"s_{sj}_{h}_{ti}"
                    )
                    nc.tensor.matmul(
                        out=ps_s[:, c0:SB],
                        lhsT=kt[:, ti * HD : (ti + 1) * HD],
                        rhs=qts_[:, c0:SB],
                        start=True,
                        stop=True,
                    )
                    es = es_pool.tile(
                        [HD, SB], BF16, tag="es", bufs=8, name=f"es_{sj}_{h}_{ti}"
                    )
                    nc.scalar.activation(
                        out=es[:, c0:SB], in_=ps_s[:, c0:SB], func=Exp,
                        scale=float(SCALE),
                    )
                    if kdiag >= 0:
                        # triangular part: first HD valid columns; block 0 is
                        # rope-congested on gpsimd, so alternate with DVE there
                        eng = nc.vector if (sj == 0 and ti % 2 == 1) else nc.gpsimd
                        eng.tensor_mul(
                            out=es[:, c0 : c0 + HD],
                            in0=es[:, c0 : c0 + HD],
                            in1=tri,
                        )
                    if ti == 0:
                        hstate[h]["es0"] = es  # acc init fused into ti=1's add
                    elif ti == 1:
                        es0 = hstate[h]["es0"]
                        nc.vector.tensor_add(
                            out=acc_[:, c0:SB], in0=es0[:, c0:SB],
                            in1=es[:, c0:SB],
                        )
                        if c0 > 0:
                            nc.vector.tensor_copy(acc_[:, 0:c0], es0[:, 0:c0])
                    else:
                        nc.vector.tensor_add(
                            out=acc_[:, c0:SB], in0=acc_[:, c0:SB],
                            in1=es[:, c0:SB],
                        )
                    return (h, ti, es, c0)

                def emit_back(item, hstate=hstate, nt=nt):
                    h, ti, es, c0 = item
                    nc.tensor.matmul(
                        out=hstate[h]["o"][:, c0:SB],
                        lhsT=v_sb[:, ti, :],
                        rhs=es[:, c0:SB],
                        start=(ti == 0),
                        stop=(ti == nt - 1),
                    )

                def make_norm(h, hstate=hstate, sj=sj, s0=s0):
                    def norm_emit():
                        # den = colsum(acc), broadcast via all-ones stationary
                        ps_den = ps.tile(
                            [HD, SB], F32, tag="den", bufs=1, name=f"den_{sj}_{h}"
                        )
                        nc.tensor.matmul(
                            out=ps_den, lhsT=ones_sb, rhs=hstate[h]["acc"],
                            start=True, stop=True,
                        )
                        rb = work.tile([HD, SB], F32, tag="rb")
                        nc.vector.reciprocal_approx_fast(out=rb, in_=ps_den)
                        nc.vector.tensor_mul(
                            out=on_sb[:, h, s0 : s0 + SB], in0=hstate[h]["o"],
                            in1=rb,
                        )
                    return norm_emit

                # flat (h, ti) pipeline: the back stream lags LOOKAHEAD tiles
                # and crosses head boundaries, so head starts have no bubble
                pend = []
                drain_until(("tp", sj))
                for h in range(NH):
                    hstate[h] = {
                        "q": q_tiles[(sj, h)],
                        "o": ps.tile([HD, SB], F32, tag="o", bufs=2,
                                     name=f"o_{sj}_{h}"),
                        "acc": es_pool.tile([HD, SB], BF16, tag="acc", bufs=2,
                                            name=f"acc_{sj}_{h}"),
                    }
                    for ti in range(nt):
                        pend.append(emit_front(h, ti))
                        if len(pend) > LOOKAHEAD:
                            emit_back(pend.pop(0))
                        if ti == 3 and deferred_norm[0] is not None:
                            deferred_norm[0]()
                            deferred_norm[0] = None
                        for _ in range(r):
                            # the last block reserves wo units to bridge the
                            # final norm's latency (a PE idle there drops the
                            # clock p-state and slows the whole wo tail)
                            pop_one(wo_floor=12 if sj == NSJ - 1 else 0)
                        # pull the q chains through early: all three pop
                        # during head 0 (PE-dense clusters; their ropes
                        # pipeline on DVE one head ahead of consumption)
                        if h == 0 and ti in (0, 1, 2):
                            drain_until(("q", sj, ti + 1))
                        elif h >= 1 and h + 1 < NH and ti == 0:
                            drain_until(("q", sj, h + 1))
                    deferred_norm[0] = make_norm(h)
                while pend:
                    emit_back(pend.pop(0))

                # cover the last head's colsum latency with a few queue pops
                for _ in range(8):
                    pop_one(wo_floor=4 if sj == NSJ - 1 else 0)
                deferred_norm[0]()
                deferred_norm[0] = None

                append_wo_block(sj)
                # keep wo backlog to stuff later attention blocks; block 3's
                # own chains are the only tail
                while len(wo_q) > WO_KEEP[sj]:
                    wo_q.pop(0)()
            while pop_one():
                pass

    nc.compile()
    return nc


def _get_program():
    global _PROG
    if _PROG is None:
        _PROG = _build_program()
    return _PROG


def _make_in_maps(x, freqs_cos, freqs_sin, wq, wk, wv, wo):
    perm = np.concatenate([np.arange(0, HD, 2), np.arange(1, HD, 2)])  # even|odd

    costT = np.ascontiguousarray(np.asarray(freqs_cos, np.float32).T).astype(F16)
    sintT = np.ascontiguousarray(np.asarray(freqs_sin, np.float32).T).astype(F16)

    tt = np.arange(HD)[:, None]
    ss = np.arange(HD)[None, :]
    tri = (tt <= ss).astype(BF)  # lower-tri in [t, s]: valid iff t <= s
    ident = np.eye(HD, dtype=BF)
    ones = np.ones((HD, HD), dtype=BF)

    # permute q/k head-dim columns so rope pairs land on partition halves
    def permute_heads(w, n_heads):
        w = np.asarray(w, np.float32).reshape(D, n_heads, HD)
        return w[:, :, perm].reshape(D, n_heads * HD)

    wq_p = permute_heads(wq, N_HEADS)
    wk_p = permute_heads(wk, N_KV_HEADS)
    wv_ = np.asarray(wv, np.float32)
    wo_ = np.asarray(wo, np.float32)
    x_ = np.asarray(x, np.float32)

    in_maps = []
    for c in range(8):
        b, g = divmod(c, 4)
        # column order [k, q0, v, q1, q2, q3]: the first 384-col half feeds
        # the interleaved block-0 prolog; q heads then arrive in use order
        wq_g = wq_p[:, g * NH * HD : (g + 1) * NH * HD]
        wqkv = np.concatenate(
            [
                wk_p[:, g * HD : (g + 1) * HD],
                wq_g[:, 0:HD],
                wv_[:, g * HD : (g + 1) * HD],
                wq_g[:, HD:],
            ],
            axis=1,
        )
        in_maps.append(
            {
                "xt": np.ascontiguousarray(x_[b].T).astype(BF),
                "wqkv": np.ascontiguousarray(wqkv).astype(BF),
                "wo": np.ascontiguousarray(
                    wo_[g * NH * HD : (g + 1) * NH * HD, :]
                ).astype(BF),
                "cost": costT,
                "sint": sintT,
                "tri": tri,
                "ident": ident,
                "ones": ones,
            }
        )
    return in_maps


def run(x, freqs_cos, freqs_sin, wq, wk, wv, wo, trace=False):
    from concourse.bass_utils import run_bass_kernel_spmd

    nc = _get_program()
    in_maps = _make_in_maps(x, freqs_cos, freqs_sin, wq, wk, wv, wo)
    res = run_bass_kernel_spmd(nc, in_maps, list(range(8)), trace=trace)
    out = np.empty((B, S, D), dtype=np.float32)
    for b in range(B):
        acc = res.results[b * 4]["y"].astype(np.float32)
        for g in range(1, 4):
            acc = acc + res.results[b * 4 + g]["y"].astype(np.float32)
        out[b] = acc
    return out, res


def kernel(x, freqs_cos, freqs_sin, wq, wk, wv, wo):
    out, _ = run(x, freqs_cos, freqs_sin, wq, wk, wv, wo, trace=False)
    return out


# revision 61
# speedup vs baseline: 1.0074x; 1.0054x over previous
"""GQA attention forward (B=2, S=2048, D=2048, 16 q heads / 4 kv heads, RoPE,
causal) on 8 Trainium2 NeuronCores.

Sharding: core c <-> (batch b = c//4, kv-group g = c%4). Each core computes its
4 query heads + 1 kv head end-to-end, including its row-shard of wo; the host
sums the 4 wo-partials per batch (the "all-reduce after wo" of the tensor
parallel scheme, done at gather time).

Layout tricks:
  - x is passed transposed (d-major) so every matmul contraction dim lands on
    SBUF partitions.
  - wq/wk columns are permuted per head (even dims -> partitions 0..63, odd ->
    64..127) so RoPE becomes plain elementwise DVE math on partition halves.
    The permutation cancels in q.k dot products.
  - all matmuls run in bf16 (fp8 DoubleRow measured exactly 2x on HW, so
    error-compensated fp8 (3 matmuls per 2 bf16-equivalents) is a net loss);
    accumulation stays fp32 in PSUM.
  - scores are built transposed ([t, s]); the softmax denominator is an
    all-ones-matrix matmul accumulated in PSUM, which lands the denominator
    already broadcast across partitions.
  - deferred-work queues (high-prio: next-block projection chains + v
    transposes; low-prio: wo chains) hold per-matmul micro-ops; the attention
    tile loop stuffs them into the PE slack left by the scalar-engine exp
    pacing (~220ns/tile). Each block's q chains are force-drained during the
    previous attention phase / head 0 so their rope latency (~4us serial
    DVE+gpsimd per chain) hides behind a full head of attention work.
    wo backlog is retained so block 3's large attention phase has stuff work,
    and a small wo reserve bridges the final norm's latency (a PE idle there
    drops the clock p-state and slows the whole wo drain tail ~630ns/matmul).
  - startup: the PE warmup (clock-gate ramp) runs on a memset tile (no DMA
    dependency, first matmul ~7.5us); block-0 x rides sync while wqkv h1 +
    fp16 cos/sin ride scalar, ordered by first use, so the dt-interleaved
    block-0 k/q0/v prolog starts on the first quarter (~13us) and streams at
    DMA arrival pace. x s-blocks 1-3 load s-block-major so block sj+1's
    projections never wait on a later quarter. y writes DMA via the
    otherwise-idle sync engine (scalar issue cost would eat exp headroom).

Measured (8 cores, core-0 profile): 262.6-263.6us; PE busy ~221us of that.
Dead ends measured on HW: fp8 DoubleRow is exactly 2x bf16 per matmul, so
error-compensated fp8 (3 matmuls per 2 bf16-equivalents) is a 1.5x net loss;
plain fp8 fails the 2e-2 gate (5.7e-2); half-width (256) attention segments
double per-op overheads and flip block 3 scalar-bound; AV LOOKAHEAD=5
corrupts numerics (es-pool lifetime); gpsimd cannot access PSUM.
"""

import ml_dtypes
import numpy as np

BF = ml_dtypes.bfloat16
F16 = np.float16
B, S, D = 2, 2048, 2048
N_HEADS, N_KV_HEADS, HD = 16, 4, 128
NH = N_HEADS // N_KV_HEADS  # q heads per core = 4
SB = 512                    # s-block (moving dim per matmul)
NSJ = S // SB               # 4 s-blocks
NT = S // HD                # 16 t-tiles (and d-tiles)
NM = NH + 2                 # 6 projection column-blocks: k, v, q0..q3
H2 = HD // 2
SCALE = 1.0 / np.sqrt(HD).astype(np.float32)

_PROG = None  # built once per process


def _build_program():
    import concourse.bacc as bacc
    import concourse.tile as tile
    from concourse import mybir

    F32 = mybir.dt.float32
    BF16 = mybir.dt.bfloat16
    FP16 = mybir.dt.float16
    Exp = mybir.ActivationFunctionType.Exp

    nc = bacc.Bacc("TRN2", target_bir_lowering=False, debug=False)

    xt_d = nc.declare_dram_parameter("xt", [D, S], BF16, isOutput=False)
    wqkv_d = nc.declare_dram_parameter("wqkv", [D, NM * HD], BF16, isOutput=False)
    wo_d = nc.declare_dram_parameter("wo", [NH * HD, D], BF16, isOutput=False)
    cost_d = nc.declare_dram_parameter("cost", [H2, S], FP16, isOutput=False)
    sint_d = nc.declare_dram_parameter("sint", [H2, S], FP16, isOutput=False)
    tri_d = nc.declare_dram_parameter("tri", [HD, HD], BF16, isOutput=False)
    ident_d = nc.declare_dram_parameter("ident", [HD, HD], BF16, isOutput=False)
    ones_d = nc.declare_dram_parameter("ones", [HD, HD], BF16, isOutput=False)
    y_d = nc.declare_dram_parameter("y", [S, D], BF16, isOutput=True)

    with tile.TileContext(nc) as tc:
        with (
            tc.tile_pool(name="consts", bufs=1) as consts,
            tc.tile_pool(name="persist", bufs=1) as persist,
            tc.tile_pool(name="work", bufs=2) as work,
            tc.tile_pool(name="xts_pool", bufs=1) as xts_pool,
            tc.tile_pool(name="qk_pool", bufs=1) as qk_pool,
            tc.tile_pool(name="es_pool", bufs=1) as es_pool,
            tc.tile_pool(name="ps", bufs=1, space="PSUM") as ps,
        ):
            tri = consts.tile([HD, HD], BF16, tag="tri")
            ident = consts.tile([HD, HD], BF16, tag="ident")
            ones_sb = consts.tile([HD, HD], BF16, tag="ones")
            cost = consts.tile([H2, S], FP16, tag="cost")
            sint = consts.tile([H2, S], FP16, tag="sint")

            wqkv = persist.tile([HD, NT, NM * HD], BF16, tag="wqkv")
            kt = persist.tile([HD, S], BF16, tag="kt")
            v_sb = persist.tile([HD, NT, HD], BF16, tag="v_sb")
            on_sb = persist.tile([HD, NH, S], BF16, tag="on")
            wo_sb = persist.tile([HD, NH, D], BF16, tag="wo")

            xt_r = xt_d[:, :].rearrange("(t p) s -> p t s", p=HD)
            wqkv_r = wqkv_d[:, :].rearrange("(t p) m -> p t m", p=HD)

            # ---- PE warm-up on a memset tile: no DMA dependency, so the
            # clock-gate (HAM) ramp starts as soon as the preamble ends ----
            dmy = consts.tile([HD, SB], BF16, tag="dmy")
            nc.vector.memset(dmy, 0.0)
            ps_warm = ps.tile([HD, SB], F32, tag="s", bufs=3, name="warmup")
            NWARM = 42
            for w in range(NWARM):
                nc.tensor.matmul(
                    out=ps_warm, lhsT=dmy[:, 0:HD], rhs=dmy,
                    start=(w == 0), stop=(w == NWARM - 1),
                )

            # ---- startup DMAs (hwdge queues: sync + scalar; gpsimd swdge
            # only for tiny consts). Ordered by first use so the interleaved
            # block-0 projection prolog can start after the first quarter;
            # two h2 quarters ride sync so all q-head weights land by ~23us ----
            MH = 3 * HD  # first column-half: k, q0, v
            xts_tiles = {}
            xrest_tiles = {}
            # sync: x block-0 quarters in dt order
            for ck in range(4):
                xq = xts_pool.tile(
                    [HD, NT // 4, SB], BF16, tag="xts", bufs=4, name=f"xts_0_{ck}"
                )
                nc.sync.dma_start(out=xq, in_=xt_r[:, ck * 4 : (ck + 1) * 4, 0:SB])
                xts_tiles[(0, ck)] = xq
            # scalar: first wqkv quarter (the PE's first real work), then
            # cos/sin (fp16, needed by rope-k ~20us), then the rest of h1
            nc.scalar.dma_start(
                out=wqkv[:, 0:4, 0:MH], in_=wqkv_r[:, 0:4, 0:MH]
            )
            nc.scalar.dma_start(out=cost, in_=cost_d[:, :])
            nc.scalar.dma_start(out=sint, in_=sint_d[:, :])
            for ck in range(1, 4):
                nc.scalar.dma_start(
                    out=wqkv[:, ck * 4 : (ck + 1) * 4, 0:MH],
                    in_=wqkv_r[:, ck * 4 : (ck + 1) * 4, 0:MH],
                )
            # preload the exp activation table while DMAs stream
            actwarm = work.tile([HD, 1], BF16, tag="actwarm", bufs=1)
            nc.scalar.activation(
                out=actwarm, in_=dmy[:, 0:1],
                func=mybir.ActivationFunctionType.Exp,
            )
            # wqkv second halves (q1..q3 columns): split scalar/sync
            for ck, eng in [(0, nc.scalar), (1, nc.scalar), (2, nc.sync), (3, nc.sync)]:
                eng.dma_start(
                    out=wqkv[:, ck * 4 : (ck + 1) * 4, MH : NM * HD],
                    in_=wqkv_r[:, ck * 4 : (ck + 1) * 4, MH : NM * HD],
                )
            nc.gpsimd.dma_start(out=tri, in_=tri_d[:, :])
            nc.gpsimd.dma_start(out=ident, in_=ident_d[:, :])
            nc.gpsimd.dma_start(out=ones_sb, in_=ones_d[:, :])
            # x s-blocks 1-3, s-block-major so earlier blocks land first
            for sj in range(1, NSJ):
                for ck in range(4):
                    xr = xts_pool.tile(
                        [HD, NT // 4, SB], BF16, tag="xrest", bufs=12,
                        name=f"xrest_{sj}_{ck}",
                    )
                    nc.sync.dma_start(
                        out=xr,
                        in_=xt_r[:, ck * 4 : (ck + 1) * 4, sj * SB : (sj + 1) * SB],
                    )
                    xrest_tiles[(sj, ck)] = xr
            nc.scalar.dma_start(
                out=wo_sb, in_=wo_d[:, :].rearrange("(h p) d -> p h d", p=HD)
            )

            def xq_ap(sj, dt):
                ck, sub = dt // 4, dt % 4
                if sj == 0:
                    return xts_tiles[(0, ck)][:, sub, :]
                return xrest_tiles[(sj, ck)][:, sub, :]

            # ---- deferred-work queues: proj (high prio) and wo (low) ----
            proj_q = []   # ('op', closure) | ('marker', key)
            wo_q = []     # closures
            passed = set()
            q_tiles = {}  # (sj, h) -> tile, filled lazily by rope closures
            vt_pending = {}

            def pop_proj():
                while proj_q:
                    kind, payload = proj_q.pop(0)
                    if kind == "marker":
                        passed.add(payload)
                        continue
                    payload()
                    return True
                return False

            def pop_one(wo_floor=0):
                if pop_proj():
                    return True
                if len(wo_q) > wo_floor:
                    wo_q.pop(0)()
                    return True
                return False

            def drain_until(marker):
                while marker not in passed and proj_q:
                    pop_proj()

            # wqkv column-block order (host-permuted to match consumption):
            # m=0: k, m=1: q0, m=2: v, m=3..5: q1..q3
            def m_to_qhead(m):
                return 0 if m == 1 else m - 2

            def rope_emit(pp, sj, m):
                # rows 0:64 = even dims (xr), 64:128 = odd (xi)
                # out_even = xr*c - xi*s ; out_odd = xr*s + xi*c
                # one PSUM->bf16 copy, then all muls run in DVE 2x mode
                # (fp32-PSUM-input ops cost 717ns vs 335ns for bf16 SBUF)
                s0 = sj * SB
                if m == 0:
                    dst = kt[:, s0 : s0 + SB]
                else:
                    h = m_to_qhead(m)
                    dst = qk_pool.tile(
                        [HD, SB], BF16, tag="qk", bufs=8, name=f"q_{sj}_{h}"
                    )
                    q_tiles[(sj, h)] = dst
                c = cost[:, s0 : s0 + SB]
                sn = sint[:, s0 : s0 + SB]
                ta = work.tile([H2, SB], F32, tag="ropeA")
                tb = work.tile([H2, SB], F32, tag="ropeB")
                nc.vector.tensor_mul(out=ta, in0=pp[0:H2, :], in1=c)
                nc.vector.tensor_mul(out=tb, in0=pp[H2:HD, :], in1=sn)
                nc.gpsimd.tensor_sub(out=dst[0:H2, :], in0=ta, in1=tb)
                tc2 = work.tile([H2, SB], F32, tag="ropeA")
                td = work.tile([H2, SB], F32, tag="ropeB")
                nc.vector.tensor_mul(out=tc2, in0=pp[0:H2, :], in1=sn)
                nc.vector.tensor_mul(out=td, in0=pp[H2:HD, :], in1=c)
                nc.gpsimd.tensor_add(out=dst[H2:HD, :], in0=tc2, in1=td)

            def proj_chain_units(sj, m):
                """16 matmul micro-ops; rope/vt handling rides the last one."""
                state = {}

                def mk(dt):
                    def f():
                        if dt == 0:
                            state["pp"] = ps.tile(
                                [HD, SB], F32, tag="pp", bufs=2, name=f"pp_{sj}_{m}"
                            )
                        nc.tensor.matmul(
                            out=state["pp"],
                            lhsT=wqkv[:, dt, m * HD : (m + 1) * HD],
                            rhs=xq_ap(sj, dt),
                            start=(dt == 0),
                            stop=(dt == NT - 1),
                        )
                        if dt == NT - 1:
                            if m == 2:
                                vt = work.tile([HD, SB], BF16, tag="vt")
                                nc.scalar.copy(out=vt, in_=state["pp"])
                                vt_pending[sj] = vt
                            else:
                                rope_emit(state["pp"], sj, m)

                    return f

                return [("op", mk(dt)) for dt in range(NT)]

            def vtp_units(sj):
                """v[t, hd] transposes for AV's stationary (4 micro-ops)."""
                units = []
                for qq in range(SB // HD):
                    def f(qq=qq):
                        pt = ps.tile(
                            [HD, HD], BF16, tag="pp", bufs=2, name=f"pt_{sj}_{qq}"
                        )
                        nc.tensor.transpose(
                            pt, vt_pending[sj][:, qq * HD : (qq + 1) * HD], ident
                        )
                        nc.scalar.copy(out=v_sb[:, sj * 4 + qq, :], in_=pt)
                    units.append(("op", f))
                return units

            def enqueue_q_chains(sj):
                for h in range(1, NH):
                    proj_q.extend(proj_chain_units(sj, 2 + h))
                    proj_q.append(("marker", ("q", sj, h)))

            def enqueue_block_proj(sj):
                """Projection of block sj as micro-ops with readiness markers:
                ("tp", sj) = k/q0/v chains + transposes emitted (attention can
                start); ("q", sj, h) = head h's q chain + rope emitted."""
                proj_q.extend(proj_chain_units(sj, 0))       # k
                proj_q.extend(proj_chain_units(sj, 1))       # q0
                proj_q.extend(proj_chain_units(sj, 2))       # v
                proj_q.extend(vtp_units(sj))
                proj_q.append(("marker", ("tp", sj)))
                enqueue_q_chains(sj)

            def prolog_block0():
                """Block-0 k/q0/v chains interleaved at dt granularity so the
                PE consumes x/wqkv quarters as the startup DMAs land (the v
                chain borrows a PSUM bank from the idle "o" tag); q1-q3 ride
                the deferred queue, pulled in by attention(0)'s head-0 forces."""
                pps = {
                    0: ps.tile([HD, SB], F32, tag="pp", bufs=2, name="pp_0_0"),
                    1: ps.tile([HD, SB], F32, tag="pp", bufs=2, name="pp_0_1"),
                    2: ps.tile([HD, SB], F32, tag="o", bufs=2, name="pp_0_2"),
                }
                # chains run SEQUENTIALLY (q0, then k, then v) so q0's chain
                # stops ~7us earlier than a dt-interleave would allow and its
                # rope (the first score's gate) starts immediately; the k/v
                # chain matmuls then overlap the rope work on DVE/gpsimd
                for m in (1, 0, 2):
                    for dt in range(NT):
                        nc.tensor.matmul(
                            out=pps[m],
                            lhsT=wqkv[:, dt, m * HD : (m + 1) * HD],
                            rhs=xq_ap(0, dt),
                            start=(dt == 0),
                            stop=(dt == NT - 1),
                        )
                    if m == 1:
                        rope_emit(pps[1], 0, 1)
                # rope-k in two s-halves so kt tiles 0-1 are ready earlier
                for lo, hi in ((0, SB // 2), (SB // 2, SB)):
                    wd = hi - lo
                    ta = work.tile([H2, SB], F32, tag="ropeA")
                    tb = work.tile([H2, SB], F32, tag="ropeB")
                    nc.vector.tensor_mul(
                        out=ta[:, 0:wd], in0=pps[0][0:H2, lo:hi], in1=cost[:, lo:hi]
                    )
                    nc.vector.tensor_mul(
                        out=tb[:, 0:wd], in0=pps[0][H2:HD, lo:hi], in1=sint[:, lo:hi]
                    )
                    nc.gpsimd.tensor_sub(
                        out=kt[0:H2, lo:hi], in0=ta[:, 0:wd], in1=tb[:, 0:wd]
                    )
                    tc2 = work.tile([H2, SB], F32, tag="ropeA")
                    td = work.tile([H2, SB], F32, tag="ropeB")
                    nc.vector.tensor_mul(
                        out=tc2[:, 0:wd], in0=pps[0][0:H2, lo:hi], in1=sint[:, lo:hi]
                    )
                    nc.vector.tensor_mul(
                        out=td[:, 0:wd], in0=pps[0][H2:HD, lo:hi], in1=cost[:, lo:hi]
                    )
                    nc.gpsimd.tensor_add(
                        out=kt[H2:HD, lo:hi], in0=tc2[:, 0:wd], in1=td[:, 0:wd]
                    )
                vt = work.tile([HD, SB], BF16, tag="vt")
                nc.scalar.copy(out=vt, in_=pps[2])
                vt_pending[0] = vt
                for kind, f in vtp_units(0):
                    f()
                passed.add(("tp", 0))
                enqueue_q_chains(0)
                # the k/q0 ropes take ~8us of serial DVE/gpsimd after the
                # chains stop; run the q1/q2 chains meanwhile so the PE
                # doesn't idle between prolog and attention(0)
                drain_until(("q", 0, 1))
                drain_until(("q", 0, 2))

            def append_wo_block(sj):
                for stl in range(4):
                    st = sj * 4 + stl
                    t0 = st * HD
                    for dj in range(NSJ):
                        state = {}
                        for hh in range(NH):
                            def f(hh=hh, dj=dj, st=st, t0=t0, state=state):
                                if hh == 0:
                                    state["ps_y"] = ps.tile(
                                        [HD, SB], F32, tag="pp", bufs=2,
                                        name=f"ps_y_{st}_{dj}",
                                    )
                                nc.tensor.matmul(
                                    out=state["ps_y"],
                                    lhsT=on_sb[:, hh, t0 : t0 + HD],
                                    rhs=wo_sb[:, hh, dj * SB : (dj + 1) * SB],
                                    start=(hh == 0),
                                    stop=(hh == NH - 1),
                                )
                                if hh == NH - 1:
                                    y_sb = work.tile(
                                        [HD, SB], BF16, tag="ysb", bufs=4,
                                        name=f"ysb_{st}_{dj}",
                                    )
                                    # scalar takes 3 of 4 copies (DVE is the
                                    # rope/acc engine); all y DMA issues ride
                                    # the idle sync engine (scalar issue cost
                                    # ~650ns each would eat exp headroom)
                                    if dj % 4 == 3:
                                        nc.vector.tensor_copy(y_sb, state["ps_y"])
                                    else:
                                        nc.scalar.copy(out=y_sb, in_=state["ps_y"])
                                    nc.sync.dma_start(
                                        out=y_d[t0 : t0 + HD, dj * SB : (dj + 1) * SB],
                                        in_=y_sb,
                                    )
                            wo_q.append(f)

            # ---- main loop ----
            prolog_block0()

            STUFF_RATE = {0: 4, 1: 5, 2: 4, 3: 3}
            WO_KEEP = {0: 64, 1: 128, 2: 128, 3: 0}

            for sj in range(NSJ):
                s0 = sj * SB
                if sj + 1 < NSJ:
                    enqueue_block_proj(sj + 1)

                nt = 4 * sj + 4  # causal: t-tiles 0..nt-1
                LOOKAHEAD = 4
                r = STUFF_RATE[sj]
                deferred_norm = [None]
                hstate = {}

                def emit_front(h, ti, hstate=hstate, sj=sj):
                    qts_, acc_ = hstate[h]["q"], hstate[h]["acc"]
                    kdiag = ti - 4 * sj
                    c0 = max(0, kdiag) * HD  # first valid column (diag band)
                    ps_s = ps.tile(
                        [HD, SB], F32, tag="s", bufs=3, name=f"s_{sj}_{h}_{ti}"
                    )
                    nc.tensor.matmul(
                        out=ps_s[:, c0:SB],
                        lhsT=kt[:, ti * HD : (ti + 1) * HD],
                        rhs=qts_[:, c0:SB],
                        start=True,
                        stop=True,
                    )
                    es = es_pool.tile(
                        [HD, SB], BF16, tag="es", bufs=8, name=f"es_{sj}_{h}_{ti}"
                    )
                    nc.scalar.activation(
                        out=es[:, c0:SB], in_=ps_s[:, c0:SB], func=Exp,
                        scale=float(SCALE),
                    )
                    if kdiag >= 0:
                        # triangular part: first HD valid columns; block 0 is
                        # rope-congested on gpsimd, so alternate with DVE there
                        eng = nc.vector if (sj == 0 and ti % 2 == 1) else nc.gpsimd
                        eng.tensor_mul(
                            out=es[:, c0 : c0 + HD],
                            in0=es[:, c0 : c0 + HD],
                            in1=tri,
                        )
                    if ti == 0:
                        hstate[h]["es0"] = es  # acc init fused into ti=1's add
                    elif ti == 1:
                        es0 = hstate[h]["es0"]
                        nc.vector.tensor_add(
                            out=acc_[:, c0:SB], in0=es0[:, c0:SB],
                            in1=es[:, c0:SB],
                        )
                        if c0 > 0:
                            nc.vector.tensor_copy(acc_[:, 0:c0], es0[:, 0:c0])
                    else:
                        nc.vector.tensor_add(
                            out=acc_[:, c0:SB], in0=acc_[:, c0:SB],
                            in1=es[:, c0:SB],
                        )
                    return (h, ti, es, c0)

                def emit_back(item, hstate=hstate, nt=nt):
                    h, ti, es, c0 = item
                    nc.tensor.matmul(
                        out=hstate[h]["o"][:, c0:SB],
                        lhsT=v_sb[:, ti, :],
                        rhs=es[:, c0:SB],
                        start=(ti == 0),
                        stop=(ti == nt - 1),
                    )

                def make_norm(h, hstate=hstate, sj=sj, s0=s0):
                    def norm_emit():
                        # den = colsum(acc), broadcast via all-ones stationary
                        ps_den = ps.tile(
                            [HD, SB], F32, tag="den", bufs=1, name=f"den_{sj}_{h}"
                        )
                        nc.tensor.matmul(
                            out=ps_den, lhsT=ones_sb, rhs=hstate[h]["acc"],
                            start=True, stop=True,
                        )
                        rb = work.tile([HD, SB], F32, tag="rb")
                        nc.vector.reciprocal_approx_fast(out=rb, in_=ps_den)
                        nc.vector.tensor_mul(
                            out=on_sb[:, h, s0 : s0 + SB], in0=hstate[h]["o"],
                            in1=rb,
                        )
                    return norm_emit

                # flat (h, ti) pipeline: the back stream lags LOOKAHEAD tiles
                # and crosses head boundaries, so head starts have no bubble
                pend = []
                drain_until(("tp", sj))
                for h in range(NH):
                    hstate[h] = {
                        "q": q_tiles[(sj, h)],
                        "o": ps.tile([HD, SB], F32, tag="o", bufs=2,
                                     name=f"o_{sj}_{h}"),
                        "acc": es_pool.tile([HD, SB], BF16, tag="acc", bufs=2,
                                            name=f"acc_{sj}_{h}"),
                    }
                    for ti in range(nt):
                        pend.append(emit_front(h, ti))
                        if len(pend) > LOOKAHEAD:
                            emit_back(pend.pop(0))
                        if ti == 3 and deferred_norm[0] is not None:
                            deferred_norm[0]()
                            deferred_norm[0] = None
                        for _ in range(r):
                            # the last block reserves wo units to bridge the
                            # final norm's latency (a PE idle there drops the
                            # clock p-state and slows the whole wo tail)
                            pop_one(wo_floor=12 if sj == NSJ - 1 else 0)
                        # pull the q chains through early: all three pop
                        # during head 0 (PE-dense clusters; their ropes
                        # pipeline on DVE one head ahead of consumption)
                        if h == 0 and ti in (0, 1, 2):
                            drain_until(("q", sj, ti + 1))
                        elif h >= 1 and h + 1 < NH and ti == 0:
                            drain_until(("q", sj, h + 1))
                    deferred_norm[0] = make_norm(h)
                while pend:
                    emit_back(pend.pop(0))

                # cover the last head's colsum latency with a few queue pops
                for _ in range(8):
                    pop_one(wo_floor=4 if sj == NSJ - 1 else 0)
                deferred_norm[0]()
                deferred_norm[0] = None

                append_wo_block(sj)
                # keep wo backlog to stuff later attention blocks; block 3's
                # own chains are the only tail
                while len(wo_q) > WO_KEEP[sj]:
                    wo_q.pop(0)()
            while pop_one():
                pass

    nc.compile()
    return nc


def _get_program():
    global _PROG
    if _PROG is None:
        _PROG = _build_program()
    return _PROG


def _make_in_maps(x, freqs_cos, freqs_sin, wq, wk, wv, wo):
    perm = np.concatenate([np.arange(0, HD, 2), np.arange(1, HD, 2)])  # even|odd

    costT = np.ascontiguousarray(np.asarray(freqs_cos, np.float32).T).astype(F16)
    sintT = np.ascontiguousarray(np.asarray(freqs_sin, np.float32).T).astype(F16)

    tt = np.arange(HD)[:, None]
    ss = np.arange(HD)[None, :]
    tri = (tt <= ss).astype(BF)  # lower-tri in [t, s]: valid iff t <= s
    ident = np.eye(HD, dtype=BF)
    ones = np.ones((HD, HD), dtype=BF)

    # permute q/k head-dim columns so rope pairs land on partition halves
    def permute_heads(w, n_heads):
        w = np.asarray(w, np.float32).reshape(D, n_heads, HD)
        return w[:, :, perm].reshape(D, n_heads * HD)

    wq_p = permute_heads(wq, N_HEADS)
    wk_p = permute_heads(wk, N_KV_HEADS)
    wv_ = np.asarray(wv, np.float32)
    wo_ = np.asarray(wo, np.float32)
    x_ = np.asarray(x, np.float32)

    in_maps = []
    for c in range(8):
        b, g = divmod(c, 4)
        # column order [k, q0, v, q1, q2, q3]: the first 384-col half feeds
        # the interleaved block-0 prolog; q heads then arrive in use order
        wq_g = wq_p[:, g * NH * HD : (g + 1) * NH * HD]
        wqkv = np.concatenate(
            [
                wk_p[:, g * HD : (g + 1) * HD],
                wq_g[:, 0:HD],
                wv_[:, g * HD : (g + 1) * HD],
                wq_g[:, HD:],
            ],
            axis=1,
        )
        in_maps.append(
            {
                "xt": np.ascontiguousarray(x_[b].T).astype(BF),
                "wqkv": np.ascontiguousarray(wqkv).astype(BF),
                "wo": np.ascontiguousarray(
                    wo_[g * NH * HD : (g + 1) * NH * HD, :]
                ).astype(BF),
                "cost": costT,
                "sint": sintT,
                "tri": tri,
                "ident": ident,
                "ones": ones,
            }
        )
    return in_maps


def run(x, freqs_cos, freqs_sin, wq, wk, wv, wo, trace=False):
    from concourse.bass_utils import run_bass_kernel_spmd

    nc = _get_program()
    in_maps = _make_in_maps(x, freqs_cos, freqs_sin, wq, wk, wv, wo)
    res = run_bass_kernel_spmd(nc, in_maps, list(range(8)), trace=trace)
    out = np.empty((B, S, D), dtype=np.float32)
    for b in range(B):
        acc = res.results[b * 4]["y"].astype(np.float32)
        for g in range(1, 4):
            acc = acc + res.results[b * 4 + g]["y"].astype(np.float32)
        out[b] = acc
    return out, res


def kernel(x, freqs_cos, freqs_sin, wq, wk, wv, wo):
    out, _ = run(x, freqs_cos, freqs_sin, wq, wk, wv, wo, trace=False)
    return out


# revision 63
# speedup vs baseline: 1.0141x; 1.0067x over previous
"""GQA attention forward (B=2, S=2048, D=2048, 16 q heads / 4 kv heads, RoPE,
causal) on 8 Trainium2 NeuronCores.

Sharding: core c <-> (batch b = c//4, kv-group g = c%4). Each core computes its
4 query heads + 1 kv head end-to-end, including its row-shard of wo; the host
sums the 4 wo-partials per batch (the "all-reduce after wo" of the tensor
parallel scheme, done at gather time).

Layout tricks:
  - x is passed transposed (d-major) so every matmul contraction dim lands on
    SBUF partitions.
  - wq/wk columns are permuted per head (even dims -> partitions 0..63, odd ->
    64..127) so RoPE becomes plain elementwise DVE math on partition halves.
    The permutation cancels in q.k dot products.
  - all matmuls run in bf16 (fp8 DoubleRow measured exactly 2x on HW, so
    error-compensated fp8 (3 matmuls per 2 bf16-equivalents) is a net loss);
    accumulation stays fp32 in PSUM.
  - scores are built transposed ([t, s]); the softmax denominator is an
    all-ones-matrix matmul accumulated in PSUM, which lands the denominator
    already broadcast across partitions.
  - deferred-work queues (high-prio: next-block projection chains + v
    transposes; low-prio: wo chains) hold per-matmul micro-ops; the attention
    tile loop stuffs them into the PE slack left by the scalar-engine exp
    pacing (~220ns/tile). Each block's q chains are force-drained during the
    previous attention phase / head 0 so their rope latency (~4us serial
    DVE+gpsimd per chain) hides behind a full head of attention work.
    wo backlog is retained so block 3's large attention phase has stuff work,
    and a small wo reserve bridges the final norm's latency (a PE idle there
    drops the clock p-state and slows the whole wo drain tail ~630ns/matmul).
  - startup: the PE warmup (clock-gate ramp) runs on a memset tile (no DMA
    dependency, first matmul ~7.5us); block-0 x rides sync while wqkv h1 +
    fp16 cos/sin ride scalar, ordered by first use, so the dt-interleaved
    block-0 k/q0/v prolog starts on the first quarter (~13us) and streams at
    DMA arrival pace. x s-blocks 1-3 load s-block-major so block sj+1's
    projections never wait on a later quarter. y writes DMA via the
    otherwise-idle sync engine (scalar issue cost would eat exp headroom).

Measured (8 cores, core-0 profile): 262.6-263.6us; PE busy ~221us of that.
Dead ends measured on HW: fp8 DoubleRow is exactly 2x bf16 per matmul, so
error-compensated fp8 (3 matmuls per 2 bf16-equivalents) is a 1.5x net loss;
plain fp8 fails the 2e-2 gate (5.7e-2); half-width (256) attention segments
double per-op overheads and flip block 3 scalar-bound; AV LOOKAHEAD=5
corrupts numerics (es-pool lifetime); gpsimd cannot access PSUM.
"""

import ml_dtypes
import numpy as np

BF = ml_dtypes.bfloat16
F16 = np.float16
B, S, D = 2, 2048, 2048
N_HEADS, N_KV_HEADS, HD = 16, 4, 128
NH = N_HEADS // N_KV_HEADS  # q heads per core = 4
SB = 512                    # s-block (moving dim per matmul)
NSJ = S // SB               # 4 s-blocks
NT = S // HD                # 16 t-tiles (and d-tiles)
NM = NH + 2                 # 6 projection column-blocks: k, v, q0..q3
H2 = HD // 2
SCALE = 1.0 / np.sqrt(HD).astype(np.float32)

_PROG = None  # built once per process


def _build_program():
    import concourse.bacc as bacc
    import concourse.tile as tile
    from concourse import mybir

    F32 = mybir.dt.float32
    BF16 = mybir.dt.bfloat16
    FP16 = mybir.dt.float16
    Exp = mybir.ActivationFunctionType.Exp

    nc = bacc.Bacc("TRN2", target_bir_lowering=False, debug=False)

    xt_d = nc.declare_dram_parameter("xt", [D, S], BF16, isOutput=False)
    wqkv_d = nc.declare_dram_parameter("wqkv", [D, NM * HD], BF16, isOutput=False)
    wo_d = nc.declare_dram_parameter("wo", [NH * HD, D], BF16, isOutput=False)
    cost_d = nc.declare_dram_parameter("cost", [H2, S], FP16, isOutput=False)
    sint_d = nc.declare_dram_parameter("sint", [H2, S], FP16, isOutput=False)
    tri_d = nc.declare_dram_parameter("tri", [HD, HD], BF16, isOutput=False)
    ident_d = nc.declare_dram_parameter("ident", [HD, HD], BF16, isOutput=False)
    ones_d = nc.declare_dram_parameter("ones", [HD, HD], BF16, isOutput=False)
    y_d = nc.declare_dram_parameter("y", [S, D], BF16, isOutput=True)

    with tile.TileContext(nc) as tc:
        with (
            tc.tile_pool(name="consts", bufs=1) as consts,
            tc.tile_pool(name="persist", bufs=1) as persist,
            tc.tile_pool(name="work", bufs=2) as work,
            tc.tile_pool(name="xts_pool", bufs=1) as xts_pool,
            tc.tile_pool(name="qk_pool", bufs=1) as qk_pool,
            tc.tile_pool(name="es_pool", bufs=1) as es_pool,
            tc.tile_pool(name="ps", bufs=1, space="PSUM") as ps,
        ):
            tri = consts.tile([HD, HD], BF16, tag="tri")
            ident = consts.tile([HD, HD], BF16, tag="ident")
            ones_sb = consts.tile([HD, HD], BF16, tag="ones")
            cost = consts.tile([H2, S], FP16, tag="cost")
            sint = consts.tile([H2, S], FP16, tag="sint")

            wqkv = persist.tile([HD, NT, NM * HD], BF16, tag="wqkv")
            kt = persist.tile([HD, S], BF16, tag="kt")
            v_sb = persist.tile([HD, NT, HD], BF16, tag="v_sb")
            on_sb = persist.tile([HD, NH, S], BF16, tag="on")
            wo_sb = persist.tile([HD, NH, D], BF16, tag="wo")

            xt_r = xt_d[:, :].rearrange("(t p) s -> p t s", p=HD)
            wqkv_r = wqkv_d[:, :].rearrange("(t p) m -> p t m", p=HD)

            # ---- PE warm-up on a memset tile: no DMA dependency, so the
            # clock-gate (HAM) ramp starts as soon as the preamble ends ----
            dmy = consts.tile([HD, SB], BF16, tag="dmy")
            nc.vector.memset(dmy, 0.0)
            ps_warm = ps.tile([HD, SB], F32, tag="s", bufs=3, name="warmup")
            NWARM = 42
            for w in range(NWARM):
                nc.tensor.matmul(
                    out=ps_warm, lhsT=dmy[:, 0:HD], rhs=dmy,
                    start=(w == 0), stop=(w == NWARM - 1),
                )

            # ---- startup DMAs (hwdge queues: sync + scalar; gpsimd swdge
            # only for tiny consts). Ordered by first use so the interleaved
            # block-0 projection prolog can start after the first quarter;
            # two h2 quarters ride sync so all q-head weights land by ~23us ----
            MH = 3 * HD  # first column-half: k, q0, v
            xts_tiles = {}
            xrest_tiles = {}
            # sync: x block-0 quarters in dt order
            for ck in range(4):
                xq = xts_pool.tile(
                    [HD, NT // 4, SB], BF16, tag="xts", bufs=4, name=f"xts_0_{ck}"
                )
                nc.sync.dma_start(out=xq, in_=xt_r[:, ck * 4 : (ck + 1) * 4, 0:SB])
                xts_tiles[(0, ck)] = xq
            # scalar: first wqkv quarter (the PE's first real work), then
            # cos/sin (fp16, needed by rope-k ~20us), then the rest of h1
            nc.scalar.dma_start(
                out=wqkv[:, 0:4, 0:MH], in_=wqkv_r[:, 0:4, 0:MH]
            )
            nc.scalar.dma_start(out=cost, in_=cost_d[:, :])
            nc.scalar.dma_start(out=sint, in_=sint_d[:, :])
            for ck in range(1, 4):
                nc.scalar.dma_start(
                    out=wqkv[:, ck * 4 : (ck + 1) * 4, 0:MH],
                    in_=wqkv_r[:, ck * 4 : (ck + 1) * 4, 0:MH],
                )
            # preload the exp activation table while DMAs stream
            actwarm = work.tile([HD, 1], BF16, tag="actwarm", bufs=1)
            nc.scalar.activation(
                out=actwarm, in_=dmy[:, 0:1],
                func=mybir.ActivationFunctionType.Exp,
            )
            # wqkv second halves (q1..q3 columns): split scalar/sync
            for ck, eng in [(0, nc.scalar), (1, nc.scalar), (2, nc.sync), (3, nc.sync)]:
                eng.dma_start(
                    out=wqkv[:, ck * 4 : (ck + 1) * 4, MH : NM * HD],
                    in_=wqkv_r[:, ck * 4 : (ck + 1) * 4, MH : NM * HD],
                )
            nc.gpsimd.dma_start(out=tri, in_=tri_d[:, :])
            nc.gpsimd.dma_start(out=ident, in_=ident_d[:, :])
            nc.gpsimd.dma_start(out=ones_sb, in_=ones_d[:, :])
            # x s-blocks 1-3, s-block-major so earlier blocks land first
            for sj in range(1, NSJ):
                for ck in range(4):
                    xr = xts_pool.tile(
                        [HD, NT // 4, SB], BF16, tag="xrest", bufs=12,
                        name=f"xrest_{sj}_{ck}",
                    )
                    nc.sync.dma_start(
                        out=xr,
                        in_=xt_r[:, ck * 4 : (ck + 1) * 4, sj * SB : (sj + 1) * SB],
                    )
                    xrest_tiles[(sj, ck)] = xr
            nc.scalar.dma_start(
                out=wo_sb, in_=wo_d[:, :].rearrange("(h p) d -> p h d", p=HD)
            )

            def xq_ap(sj, dt):
                ck, sub = dt // 4, dt % 4
                if sj == 0:
                    return xts_tiles[(0, ck)][:, sub, :]
                return xrest_tiles[(sj, ck)][:, sub, :]

            # ---- deferred-work queues: proj (high prio) and wo (low) ----
            proj_q = []   # ('op', closure) | ('marker', key)
            wo_q = []     # closures
            passed = set()
            q_tiles = {}  # (sj, h) -> tile, filled lazily by rope closures
            vt_pending = {}

            def pop_proj():
                while proj_q:
                    kind, payload = proj_q.pop(0)
                    if kind == "marker":
                        passed.add(payload)
                        continue
                    payload()
                    return True
                return False

            def pop_one(wo_floor=0):
                if pop_proj():
                    return True
                if len(wo_q) > wo_floor:
                    wo_q.pop(0)()
                    return True
                return False

            def drain_until(marker):
                while marker not in passed and proj_q:
                    pop_proj()

            # wqkv column-block order (host-permuted to match consumption):
            # m=0: k, m=1: q0, m=2: v, m=3..5: q1..q3
            def m_to_qhead(m):
                return 0 if m == 1 else m - 2

            def rope_emit(pp, sj, m):
                # rows 0:64 = even dims (xr), 64:128 = odd (xi)
                # out_even = xr*c - xi*s ; out_odd = xr*s + xi*c
                # one PSUM->bf16 copy, then all muls run in DVE 2x mode
                # (fp32-PSUM-input ops cost 717ns vs 335ns for bf16 SBUF)
                s0 = sj * SB
                if m == 0:
                    dst = kt[:, s0 : s0 + SB]
                else:
                    h = m_to_qhead(m)
                    dst = qk_pool.tile(
                        [HD, SB], BF16, tag="qk", bufs=8, name=f"q_{sj}_{h}"
                    )
                    q_tiles[(sj, h)] = dst
                c = cost[:, s0 : s0 + SB]
                sn = sint[:, s0 : s0 + SB]
                ta = work.tile([H2, SB], F32, tag="ropeA")
                tb = work.tile([H2, SB], F32, tag="ropeB")
                nc.vector.tensor_mul(out=ta, in0=pp[0:H2, :], in1=c)
                nc.vector.tensor_mul(out=tb, in0=pp[H2:HD, :], in1=sn)
                nc.gpsimd.tensor_sub(out=dst[0:H2, :], in0=ta, in1=tb)
                tc2 = work.tile([H2, SB], F32, tag="ropeA")
                td = work.tile([H2, SB], F32, tag="ropeB")
                nc.vector.tensor_mul(out=tc2, in0=pp[0:H2, :], in1=sn)
                nc.vector.tensor_mul(out=td, in0=pp[H2:HD, :], in1=c)
                # odd-half combine on DVE: 717ns vs 1262ns on gpsimd, and it
                # rides the same queue as the muls, so the rope's critical
                # tail loses the cross-engine hop + tri-mul queueing delays
                nc.vector.tensor_add(out=dst[H2:HD, :], in0=tc2, in1=td)

            def proj_chain_units(sj, m):
                """16 matmul micro-ops; rope/vt handling rides the last one."""
                state = {}

                def mk(dt):
                    def f():
                        if dt == 0:
                            state["pp"] = ps.tile(
                                [HD, SB], F32, tag="pp", bufs=2, name=f"pp_{sj}_{m}"
                            )
                        nc.tensor.matmul(
                            out=state["pp"],
                            lhsT=wqkv[:, dt, m * HD : (m + 1) * HD],
                            rhs=xq_ap(sj, dt),
                            start=(dt == 0),
                            stop=(dt == NT - 1),
                        )
                        if dt == NT - 1:
                            if m == 2:
                                vt = work.tile([HD, SB], BF16, tag="vt")
                                nc.scalar.copy(out=vt, in_=state["pp"])
                                vt_pending[sj] = vt
                            else:
                                rope_emit(state["pp"], sj, m)

                    return f

                return [("op", mk(dt)) for dt in range(NT)]

            def vtp_units(sj):
                """v[t, hd] transposes for AV's stationary (4 micro-ops)."""
                units = []
                for qq in range(SB // HD):
                    def f(qq=qq):
                        pt = ps.tile(
                            [HD, HD], BF16, tag="pp", bufs=2, name=f"pt_{sj}_{qq}"
                        )
                        nc.tensor.transpose(
                            pt, vt_pending[sj][:, qq * HD : (qq + 1) * HD], ident
                        )
                        nc.scalar.copy(out=v_sb[:, sj * 4 + qq, :], in_=pt)
                    units.append(("op", f))
                return units

            def enqueue_q_chains(sj):
                for h in range(1, NH):
                    proj_q.extend(proj_chain_units(sj, 2 + h))
                    proj_q.append(("marker", ("q", sj, h)))

            def enqueue_block_proj(sj):
                """Projection of block sj as micro-ops with readiness markers:
                ("tp", sj) = k/q0/v chains + transposes emitted (attention can
                start); ("q", sj, h) = head h's q chain + rope emitted."""
                proj_q.extend(proj_chain_units(sj, 0))       # k
                proj_q.extend(proj_chain_units(sj, 1))       # q0
                proj_q.extend(proj_chain_units(sj, 2))       # v
                proj_q.extend(vtp_units(sj))
                proj_q.append(("marker", ("tp", sj)))
                enqueue_q_chains(sj)

            def prolog_block0():
                """Block-0 k/q0/v chains interleaved at dt granularity so the
                PE consumes x/wqkv quarters as the startup DMAs land (the v
                chain borrows a PSUM bank from the idle "o" tag); q1-q3 ride
                the deferred queue, pulled in by attention(0)'s head-0 forces."""
                pps = {
                    0: ps.tile([HD, SB], F32, tag="pp", bufs=2, name="pp_0_0"),
                    1: ps.tile([HD, SB], F32, tag="pp", bufs=2, name="pp_0_1"),
                    2: ps.tile([HD, SB], F32, tag="o", bufs=2, name="pp_0_2"),
                }
                # chains run SEQUENTIALLY (q0, then k, then v) so q0's chain
                # stops ~7us earlier than a dt-interleave would allow and its
                # rope (the first score's gate) starts immediately; the k/v
                # chain matmuls then overlap the rope work on DVE/gpsimd
                for m in (1, 0, 2):
                    for dt in range(NT):
                        nc.tensor.matmul(
                            out=pps[m],
                            lhsT=wqkv[:, dt, m * HD : (m + 1) * HD],
                            rhs=xq_ap(0, dt),
                            start=(dt == 0),
                            stop=(dt == NT - 1),
                        )
                    if m == 1:
                        rope_emit(pps[1], 0, 1)
                # rope-k in two s-halves so kt tiles 0-1 are ready earlier
                for lo, hi in ((0, SB // 2), (SB // 2, SB)):
                    wd = hi - lo
                    ta = work.tile([H2, SB], F32, tag="ropeA")
                    tb = work.tile([H2, SB], F32, tag="ropeB")
                    nc.vector.tensor_mul(
                        out=ta[:, 0:wd], in0=pps[0][0:H2, lo:hi], in1=cost[:, lo:hi]
                    )
                    nc.vector.tensor_mul(
                        out=tb[:, 0:wd], in0=pps[0][H2:HD, lo:hi], in1=sint[:, lo:hi]
                    )
                    nc.gpsimd.tensor_sub(
                        out=kt[0:H2, lo:hi], in0=ta[:, 0:wd], in1=tb[:, 0:wd]
                    )
                    tc2 = work.tile([H2, SB], F32, tag="ropeA")
                    td = work.tile([H2, SB], F32, tag="ropeB")
                    nc.vector.tensor_mul(
                        out=tc2[:, 0:wd], in0=pps[0][0:H2, lo:hi], in1=sint[:, lo:hi]
                    )
                    nc.vector.tensor_mul(
                        out=td[:, 0:wd], in0=pps[0][H2:HD, lo:hi], in1=cost[:, lo:hi]
                    )
                    nc.vector.tensor_add(
                        out=kt[H2:HD, lo:hi], in0=tc2[:, 0:wd], in1=td[:, 0:wd]
                    )
                vt = work.tile([HD, SB], BF16, tag="vt")
                nc.scalar.copy(out=vt, in_=pps[2])
                vt_pending[0] = vt
                for kind, f in vtp_units(0):
                    f()
                passed.add(("tp", 0))
                enqueue_q_chains(0)
                # the k/q0 ropes take ~8us of serial DVE/gpsimd after the
                # chains stop; run the q1/q2 chains meanwhile so the PE
                # doesn't idle between prolog and attention(0)
                drain_until(("q", 0, 1))
                drain_until(("q", 0, 2))

            def append_wo_block(sj):
                for stl in range(4):
                    st = sj * 4 + stl
                    t0 = st * HD
                    for dj in range(NSJ):
                        state = {}
                        for hh in range(NH):
                            def f(hh=hh, dj=dj, st=st, t0=t0, state=state):
                                if hh == 0:
                                    state["ps_y"] = ps.tile(
                                        [HD, SB], F32, tag="pp", bufs=2,
                                        name=f"ps_y_{st}_{dj}",
                                    )
                                nc.tensor.matmul(
                                    out=state["ps_y"],
                                    lhsT=on_sb[:, hh, t0 : t0 + HD],
                                    rhs=wo_sb[:, hh, dj * SB : (dj + 1) * SB],
                                    start=(hh == 0),
                                    stop=(hh == NH - 1),
                                )
                                if hh == NH - 1:
                                    y_sb = work.tile(
                                        [HD, SB], BF16, tag="ysb", bufs=4,
                                        name=f"ysb_{st}_{dj}",
                                    )
                                    # scalar takes 3 of 4 copies (DVE is the
                                    # rope/acc engine); all y DMA issues ride
                                    # the idle sync engine (scalar issue cost
                                    # ~650ns each would eat exp headroom)
                                    if dj % 4 == 3:
                                        nc.vector.tensor_copy(y_sb, state["ps_y"])
                                    else:
                                        nc.scalar.copy(out=y_sb, in_=state["ps_y"])
                                    nc.sync.dma_start(
                                        out=y_d[t0 : t0 + HD, dj * SB : (dj + 1) * SB],
                                        in_=y_sb,
                                    )
                            wo_q.append(f)

            # ---- main loop ----
            prolog_block0()

            STUFF_RATE = {0: 4, 1: 5, 2: 4, 3: 3}
            WO_KEEP = {0: 64, 1: 128, 2: 128, 3: 0}

            for sj in range(NSJ):
                s0 = sj * SB
                if sj + 1 < NSJ:
                    enqueue_block_proj(sj + 1)

                nt = 4 * sj + 4  # causal: t-tiles 0..nt-1
                LOOKAHEAD = 4
                r = STUFF_RATE[sj]
                deferred_norm = [None]
                hstate = {}

                def emit_front(h, ti, hstate=hstate, sj=sj):
                    qts_, acc_ = hstate[h]["q"], hstate[h]["acc"]
                    kdiag = ti - 4 * sj
                    c0 = max(0, kdiag) * HD  # first valid column (diag band)
                    ps_s = ps.tile(
                        [HD, SB], F32, tag="s", bufs=3, name=f"s_{sj}_{h}_{ti}"
                    )
                    nc.tensor.matmul(
                        out=ps_s[:, c0:SB],
                        lhsT=kt[:, ti * HD : (ti + 1) * HD],
                        rhs=qts_[:, c0:SB],
                        start=True,
                        stop=True,
                    )
                    es = es_pool.tile(
                        [HD, SB], BF16, tag="es", bufs=8, name=f"es_{sj}_{h}_{ti}"
                    )
                    nc.scalar.activation(
                        out=es[:, c0:SB], in_=ps_s[:, c0:SB], func=Exp,
                        scale=float(SCALE),
                    )
                    if kdiag >= 0:
                        # triangular part: first HD valid columns; block 0 is
                        # rope-congested on gpsimd, so alternate with DVE there
                        eng = nc.vector if (sj == 0 and ti % 2 == 1) else nc.gpsimd
                        eng.tensor_mul(
                            out=es[:, c0 : c0 + HD],
                            in0=es[:, c0 : c0 + HD],
                            in1=tri,
                        )
                    if ti == 0:
                        hstate[h]["es0"] = es  # acc init fused into ti=1's add
                    elif ti == 1:
                        es0 = hstate[h]["es0"]
                        nc.vector.tensor_add(
                            out=acc_[:, c0:SB], in0=es0[:, c0:SB],
                            in1=es[:, c0:SB],
                        )
                        if c0 > 0:
                            nc.vector.tensor_copy(acc_[:, 0:c0], es0[:, 0:c0])
                    else:
                        nc.vector.tensor_add(
                            out=acc_[:, c0:SB], in0=acc_[:, c0:SB],
                            in1=es[:, c0:SB],
                        )
                    return (h, ti, es, c0)

                def emit_back(item, hstate=hstate, nt=nt):
                    h, ti, es, c0 = item
                    nc.tensor.matmul(
                        out=hstate[h]["o"][:, c0:SB],
                        lhsT=v_sb[:, ti, :],
                        rhs=es[:, c0:SB],
                        start=(ti == 0),
                        stop=(ti == nt - 1),
                    )

                def make_norm(h, hstate=hstate, sj=sj, s0=s0):
                    def norm_emit():
                        # den = colsum(acc), broadcast via all-ones stationary
                        ps_den = ps.tile(
                            [HD, SB], F32, tag="den", bufs=1, name=f"den_{sj}_{h}"
                        )
                        nc.tensor.matmul(
                            out=ps_den, lhsT=ones_sb, rhs=hstate[h]["acc"],
                            start=True, stop=True,
                        )
                        rb = work.tile([HD, SB], F32, tag="rb")
                        nc.vector.reciprocal_approx_fast(out=rb, in_=ps_den)
                        nc.vector.tensor_mul(
                            out=on_sb[:, h, s0 : s0 + SB], in0=hstate[h]["o"],
                            in1=rb,
                        )
                    return norm_emit

                # flat (h, ti) pipeline: the back stream lags LOOKAHEAD tiles
                # and crosses head boundaries, so head starts have no bubble
                pend = []
                drain_until(("tp", sj))
                for h in range(NH):
                    hstate[h] = {
                        "q": q_tiles[(sj, h)],
                        "o": ps.tile([HD, SB], F32, tag="o", bufs=2,
                                     name=f"o_{sj}_{h}"),
                        "acc": es_pool.tile([HD, SB], BF16, tag="acc", bufs=2,
                                            name=f"acc_{sj}_{h}"),
                    }
                    for ti in range(nt):
                        pend.append(emit_front(h, ti))
                        if len(pend) > LOOKAHEAD:
                            emit_back(pend.pop(0))
                        if ti == 3 and deferred_norm[0] is not None:
                            deferred_norm[0]()
                            deferred_norm[0] = None
                        for _ in range(r):
                            # the last block reserves wo units to bridge the
                            # final norm's latency (a PE idle there drops the
                            # clock p-state and slows the whole wo tail)
                            pop_one(wo_floor=12 if sj == NSJ - 1 else 0)
                        # pull the q chains through early: all three pop
                        # during head 0 (PE-dense clusters; their ropes
                        # pipeline on DVE one head ahead of consumption)
                        if h == 0 and ti in (0, 1, 2):
                            drain_until(("q", sj, ti + 1))
                        elif h >= 1 and h + 1 < NH and ti == 0:
                            drain_until(("q", sj, h + 1))
                    deferred_norm[0] = make_norm(h)
                while pend:
                    emit_back(pend.pop(0))

                # cover the last head's colsum latency with a few queue pops
                for _ in range(8):
                    pop_one(wo_floor=4 if sj == NSJ - 1 else 0)
                deferred_norm[0]()
                deferred_norm[0] = None

                append_wo_block(sj)
                # keep wo backlog to stuff later attention blocks; block 3's
                # own chains are the only tail
                while len(wo_q) > WO_KEEP[sj]:
                    wo_q.pop(0)()
            while pop_one():
                pass

    nc.compile()
    return nc


def _get_program():
    global _PROG
    if _PROG is None:
        _PROG = _build_program()
    return _PROG


def _make_in_maps(x, freqs_cos, freqs_sin, wq, wk, wv, wo):
    perm = np.concatenate([np.arange(0, HD, 2), np.arange(1, HD, 2)])  # even|odd

    costT = np.ascontiguousarray(np.asarray(freqs_cos, np.float32).T).astype(F16)
    sintT = np.ascontiguousarray(np.asarray(freqs_sin, np.float32).T).astype(F16)

    tt = np.arange(HD)[:, None]
    ss = np.arange(HD)[None, :]
    tri = (tt <= ss).astype(BF)  # lower-tri in [t, s]: valid iff t <= s
    ident = np.eye(HD, dtype=BF)
    ones = np.ones((HD, HD), dtype=BF)

    # permute q/k head-dim columns so rope pairs land on partition halves
    def permute_heads(w, n_heads):
        w = np.asarray(w, np.float32).reshape(D, n_heads, HD)
        return w[:, :, perm].reshape(D, n_heads * HD)

    wq_p = permute_heads(wq, N_HEADS)
    wk_p = permute_heads(wk, N_KV_HEADS)
    wv_ = np.asarray(wv, np.float32)
    wo_ = np.asarray(wo, np.float32)
    x_ = np.asarray(x, np.float32)

    in_maps = []
    for c in range(8):
        b, g = divmod(c, 4)
        # column order [k, q0, v, q1, q2, q3]: the first 384-col half feeds
        # the interleaved block-0 prolog; q heads then arrive in use order
        wq_g = wq_p[:, g * NH * HD : (g + 1) * NH * HD]
        wqkv = np.concatenate(
            [
                wk_p[:, g * HD : (g + 1) * HD],
                wq_g[:, 0:HD],
                wv_[:, g * HD : (g + 1) * HD],
                wq_g[:, HD:],
            ],
            axis=1,
        )
        in_maps.append(
            {
                "xt": np.ascontiguousarray(x_[b].T).astype(BF),
                "wqkv": np.ascontiguousarray(wqkv).astype(BF),
                "wo": np.ascontiguousarray(
                    wo_[g * NH * HD : (g + 1) * NH * HD, :]
                ).astype(BF),
                "cost": costT,
                "sint": sintT,
                "tri": tri,
                "ident": ident,
                "ones": ones,
            }
        )
    return in_maps


def run(x, freqs_cos, freqs_sin, wq, wk, wv, wo, trace=False):
    from concourse.bass_utils import run_bass_kernel_spmd

    nc = _get_program()
    in_maps = _make_in_maps(x, freqs_cos, freqs_sin, wq, wk, wv, wo)
    res = run_bass_kernel_spmd(nc, in_maps, list(range(8)), trace=trace)
    out = np.empty((B, S, D), dtype=np.float32)
    for b in range(B):
        acc = res.results[b * 4]["y"].astype(np.float32)
        for g in range(1, 4):
            acc = acc + res.results[b * 4 + g]["y"].astype(np.float32)
        out[b] = acc
    return out, res


def kernel(x, freqs_cos, freqs_sin, wq, wk, wv, wo):
    out, _ = run(x, freqs_cos, freqs_sin, wq, wk, wv, wo, trace=False)
    return out


# revision 64
# speedup vs baseline: 1.0205x; 1.0063x over previous
"""GQA attention forward (B=2, S=2048, D=2048, 16 q heads / 4 kv heads, RoPE,
causal) on 8 Trainium2 NeuronCores.

Sharding: core c <-> (batch b = c//4, kv-group g = c%4). Each core computes its
4 query heads + 1 kv head end-to-end, including its row-shard of wo; the host
sums the 4 wo-partials per batch (the "all-reduce after wo" of the tensor
parallel scheme, done at gather time).

Layout tricks:
  - x is passed transposed (d-major) so every matmul contraction dim lands on
    SBUF partitions.
  - wq/wk columns are permuted per head (even dims -> partitions 0..63, odd ->
    64..127) so RoPE becomes plain elementwise DVE math on partition halves.
    The permutation cancels in q.k dot products.
  - all matmuls run in bf16 (fp8 DoubleRow measured exactly 2x on HW, so
    error-compensated fp8 (3 matmuls per 2 bf16-equivalents) is a net loss);
    accumulation stays fp32 in PSUM.
  - scores are built transposed ([t, s]); the softmax denominator is an
    all-ones-matrix matmul accumulated in PSUM, which lands the denominator
    already broadcast across partitions.
  - deferred-work queues (high-prio: next-block projection chains + v
    transposes; low-prio: wo chains) hold per-matmul micro-ops; the attention
    tile loop stuffs them into the PE slack left by the scalar-engine exp
    pacing (~220ns/tile). Each block's q chains are force-drained during the
    previous attention phase / head 0 so their rope latency (~4us serial
    DVE+gpsimd per chain) hides behind a full head of attention work.
    wo backlog is retained so block 3's large attention phase has stuff work,
    and a small wo reserve bridges the final norm's latency (a PE idle there
    drops the clock p-state and slows the whole wo drain tail ~630ns/matmul).
  - startup: the PE warmup (clock-gate ramp) runs on a memset tile (no DMA
    dependency, first matmul ~7.5us); block-0 x rides sync while wqkv h1 +
    fp16 cos/sin ride scalar, ordered by first use, so the dt-interleaved
    block-0 k/q0/v prolog starts on the first quarter (~13us) and streams at
    DMA arrival pace. x s-blocks 1-3 load s-block-major so block sj+1's
    projections never wait on a later quarter. y writes DMA via the
    otherwise-idle sync engine (scalar issue cost would eat exp headroom).

Measured (8 cores, core-0 profile): 262.6-263.6us; PE busy ~221us of that.
Dead ends measured on HW: fp8 DoubleRow is exactly 2x bf16 per matmul, so
error-compensated fp8 (3 matmuls per 2 bf16-equivalents) is a 1.5x net loss;
plain fp8 fails the 2e-2 gate (5.7e-2); half-width (256) attention segments
double per-op overheads and flip block 3 scalar-bound; AV LOOKAHEAD=5
corrupts numerics (es-pool lifetime); gpsimd cannot access PSUM.
"""

import ml_dtypes
import numpy as np

BF = ml_dtypes.bfloat16
F16 = np.float16
B, S, D = 2, 2048, 2048
N_HEADS, N_KV_HEADS, HD = 16, 4, 128
NH = N_HEADS // N_KV_HEADS  # q heads per core = 4
SB = 512                    # s-block (moving dim per matmul)
NSJ = S // SB               # 4 s-blocks
NT = S // HD                # 16 t-tiles (and d-tiles)
NM = NH + 2                 # 6 projection column-blocks: k, v, q0..q3
H2 = HD // 2
SCALE = 1.0 / np.sqrt(HD).astype(np.float32)

_PROG = None  # built once per process


def _build_program():
    import concourse.bacc as bacc
    import concourse.tile as tile
    from concourse import mybir

    F32 = mybir.dt.float32
    BF16 = mybir.dt.bfloat16
    FP16 = mybir.dt.float16
    Exp = mybir.ActivationFunctionType.Exp

    nc = bacc.Bacc("TRN2", target_bir_lowering=False, debug=False)

    xt_d = nc.declare_dram_parameter("xt", [D, S], BF16, isOutput=False)
    wqkv_d = nc.declare_dram_parameter("wqkv", [D, NM * HD], BF16, isOutput=False)
    wo_d = nc.declare_dram_parameter("wo", [NH * HD, D], BF16, isOutput=False)
    cost_d = nc.declare_dram_parameter("cost", [H2, S], FP16, isOutput=False)
    sint_d = nc.declare_dram_parameter("sint", [H2, S], FP16, isOutput=False)
    tri_d = nc.declare_dram_parameter("tri", [HD, HD], BF16, isOutput=False)
    ident_d = nc.declare_dram_parameter("ident", [HD, HD], BF16, isOutput=False)
    ones_d = nc.declare_dram_parameter("ones", [HD, HD], BF16, isOutput=False)
    y_d = nc.declare_dram_parameter("y", [S, D], BF16, isOutput=True)

    with tile.TileContext(nc) as tc:
        with (
            tc.tile_pool(name="consts", bufs=1) as consts,
            tc.tile_pool(name="persist", bufs=1) as persist,
            tc.tile_pool(name="work", bufs=2) as work,
            tc.tile_pool(name="xts_pool", bufs=1) as xts_pool,
            tc.tile_pool(name="qk_pool", bufs=1) as qk_pool,
            tc.tile_pool(name="es_pool", bufs=1) as es_pool,
            tc.tile_pool(name="ps", bufs=1, space="PSUM") as ps,
        ):
            tri = consts.tile([HD, HD], BF16, tag="tri")
            ident = consts.tile([HD, HD], BF16, tag="ident")
            ones_sb = consts.tile([HD, HD], BF16, tag="ones")
            cost = consts.tile([H2, S], FP16, tag="cost")
            sint = consts.tile([H2, S], FP16, tag="sint")

            wqkv = persist.tile([HD, NT, NM * HD], BF16, tag="wqkv")
            kt = persist.tile([HD, S], BF16, tag="kt")
            v_sb = persist.tile([HD, NT, HD], BF16, tag="v_sb")
            on_sb = persist.tile([HD, NH, S], BF16, tag="on")
            wo_sb = persist.tile([HD, NH, D], BF16, tag="wo")

            xt_r = xt_d[:, :].rearrange("(t p) s -> p t s", p=HD)
            wqkv_r = wqkv_d[:, :].rearrange("(t p) m -> p t m", p=HD)

            # ---- PE warm-up on a memset tile: no DMA dependency, so the
            # clock-gate (HAM) ramp starts as soon as the preamble ends ----
            dmy = consts.tile([HD, SB], BF16, tag="dmy")
            nc.vector.memset(dmy, 0.0)
            ps_warm = ps.tile([HD, SB], F32, tag="s", bufs=3, name="warmup")
            NWARM = 42
            for w in range(NWARM):
                nc.tensor.matmul(
                    out=ps_warm, lhsT=dmy[:, 0:HD], rhs=dmy,
                    start=(w == 0), stop=(w == NWARM - 1),
                )

            # ---- startup DMAs (hwdge queues: sync + scalar; gpsimd swdge
            # only for tiny consts). Ordered by first use so the interleaved
            # block-0 projection prolog can start after the first quarter;
            # two h2 quarters ride sync so all q-head weights land by ~23us ----
            MH = 3 * HD  # first column-half: k, q0, v
            xts_tiles = {}
            xrest_tiles = {}
            # sync: x block-0 quarters in dt order
            for ck in range(4):
                xq = xts_pool.tile(
                    [HD, NT // 4, SB], BF16, tag="xts", bufs=4, name=f"xts_0_{ck}"
                )
                nc.sync.dma_start(out=xq, in_=xt_r[:, ck * 4 : (ck + 1) * 4, 0:SB])
                xts_tiles[(0, ck)] = xq
            # scalar: first wqkv quarter (the PE's first real work), then
            # cos/sin (fp16, needed by rope-k ~20us), then the rest of h1
            nc.scalar.dma_start(
                out=wqkv[:, 0:4, 0:MH], in_=wqkv_r[:, 0:4, 0:MH]
            )
            nc.scalar.dma_start(out=cost, in_=cost_d[:, :])
            nc.scalar.dma_start(out=sint, in_=sint_d[:, :])
            for ck in range(1, 4):
                nc.scalar.dma_start(
                    out=wqkv[:, ck * 4 : (ck + 1) * 4, 0:MH],
                    in_=wqkv_r[:, ck * 4 : (ck + 1) * 4, 0:MH],
                )
            # preload the exp activation table while DMAs stream
            actwarm = work.tile([HD, 1], BF16, tag="actwarm", bufs=1)
            nc.scalar.activation(
                out=actwarm, in_=dmy[:, 0:1],
                func=mybir.ActivationFunctionType.Exp,
            )
            # wqkv second halves (q1..q3 columns): split scalar/sync
            for ck, eng in [(0, nc.scalar), (1, nc.scalar), (2, nc.sync), (3, nc.sync)]:
                eng.dma_start(
                    out=wqkv[:, ck * 4 : (ck + 1) * 4, MH : NM * HD],
                    in_=wqkv_r[:, ck * 4 : (ck + 1) * 4, MH : NM * HD],
                )
            nc.gpsimd.dma_start(out=tri, in_=tri_d[:, :])
            nc.gpsimd.dma_start(out=ident, in_=ident_d[:, :])
            nc.gpsimd.dma_start(out=ones_sb, in_=ones_d[:, :])
            # x s-blocks 1-3, s-block-major so earlier blocks land first
            for sj in range(1, NSJ):
                for ck in range(4):
                    xr = xts_pool.tile(
                        [HD, NT // 4, SB], BF16, tag="xrest", bufs=12,
                        name=f"xrest_{sj}_{ck}",
                    )
                    nc.sync.dma_start(
                        out=xr,
                        in_=xt_r[:, ck * 4 : (ck + 1) * 4, sj * SB : (sj + 1) * SB],
                    )
                    xrest_tiles[(sj, ck)] = xr
            nc.scalar.dma_start(
                out=wo_sb, in_=wo_d[:, :].rearrange("(h p) d -> p h d", p=HD)
            )

            def xq_ap(sj, dt):
                ck, sub = dt // 4, dt % 4
                if sj == 0:
                    return xts_tiles[(0, ck)][:, sub, :]
                return xrest_tiles[(sj, ck)][:, sub, :]

            # ---- deferred-work queues: proj (high prio) and wo (low) ----
            proj_q = []   # ('op', closure) | ('marker', key)
            wo_q = []     # closures
            passed = set()
            q_tiles = {}  # (sj, h) -> tile, filled lazily by rope closures
            vt_pending = {}

            def pop_proj():
                while proj_q:
                    kind, payload = proj_q.pop(0)
                    if kind == "marker":
                        passed.add(payload)
                        continue
                    payload()
                    return True
                return False

            def pop_one(wo_floor=0):
                if pop_proj():
                    return True
                if len(wo_q) > wo_floor:
                    wo_q.pop(0)()
                    return True
                return False

            def drain_until(marker):
                while marker not in passed and proj_q:
                    pop_proj()

            # wqkv column-block order (host-permuted to match consumption):
            # m=0: k, m=1: q0, m=2: v, m=3..5: q1..q3
            def m_to_qhead(m):
                return 0 if m == 1 else m - 2

            def rope_emit(pp, sj, m):
                # rows 0:64 = even dims (xr), 64:128 = odd (xi)
                # out_even = xr*c - xi*s ; out_odd = xr*s + xi*c
                # one PSUM->bf16 copy, then all muls run in DVE 2x mode
                # (fp32-PSUM-input ops cost 717ns vs 335ns for bf16 SBUF)
                s0 = sj * SB
                if m == 0:
                    dst = kt[:, s0 : s0 + SB]
                else:
                    h = m_to_qhead(m)
                    dst = qk_pool.tile(
                        [HD, SB], BF16, tag="qk", bufs=8, name=f"q_{sj}_{h}"
                    )
                    q_tiles[(sj, h)] = dst
                c = cost[:, s0 : s0 + SB]
                sn = sint[:, s0 : s0 + SB]
                ta = work.tile([H2, SB], F32, tag="ropeA")
                tb = work.tile([H2, SB], F32, tag="ropeB")
                nc.vector.tensor_mul(out=ta, in0=pp[0:H2, :], in1=c)
                nc.vector.tensor_mul(out=tb, in0=pp[H2:HD, :], in1=sn)
                nc.gpsimd.tensor_sub(out=dst[0:H2, :], in0=ta, in1=tb)
                tc2 = work.tile([H2, SB], F32, tag="ropeA")
                td = work.tile([H2, SB], F32, tag="ropeB")
                nc.vector.tensor_mul(out=tc2, in0=pp[0:H2, :], in1=sn)
                nc.vector.tensor_mul(out=td, in0=pp[H2:HD, :], in1=c)
                # odd-half combine on DVE: 717ns vs 1262ns on gpsimd, and it
                # rides the same queue as the muls, so the rope's critical
                # tail loses the cross-engine hop + tri-mul queueing delays
                nc.vector.tensor_add(out=dst[H2:HD, :], in0=tc2, in1=td)

            def proj_chain_units(sj, m):
                """16 matmul micro-ops; rope/vt handling rides the last one."""
                state = {}

                def mk(dt):
                    def f():
                        if dt == 0:
                            state["pp"] = ps.tile(
                                [HD, SB], F32, tag="pp", bufs=2, name=f"pp_{sj}_{m}"
                            )
                        nc.tensor.matmul(
                            out=state["pp"],
                            lhsT=wqkv[:, dt, m * HD : (m + 1) * HD],
                            rhs=xq_ap(sj, dt),
                            start=(dt == 0),
                            stop=(dt == NT - 1),
                        )
                        if dt == NT - 1:
                            if m == 2:
                                vt = work.tile([HD, SB], BF16, tag="vt")
                                nc.scalar.copy(out=vt, in_=state["pp"])
                                vt_pending[sj] = vt
                            else:
                                rope_emit(state["pp"], sj, m)

                    return f

                return [("op", mk(dt)) for dt in range(NT)]

            def vtp_units(sj):
                """v[t, hd] transposes for AV's stationary (4 micro-ops)."""
                units = []
                for qq in range(SB // HD):
                    def f(qq=qq):
                        pt = ps.tile(
                            [HD, HD], BF16, tag="pp", bufs=2, name=f"pt_{sj}_{qq}"
                        )
                        nc.tensor.transpose(
                            pt, vt_pending[sj][:, qq * HD : (qq + 1) * HD], ident
                        )
                        nc.scalar.copy(out=v_sb[:, sj * 4 + qq, :], in_=pt)
                    units.append(("op", f))
                return units

            def enqueue_q_chains(sj):
                for h in range(1, NH):
                    proj_q.extend(proj_chain_units(sj, 2 + h))
                    proj_q.append(("marker", ("q", sj, h)))

            def enqueue_block_proj(sj):
                """Projection of block sj as micro-ops with readiness markers:
                ("tp", sj) = k/q0/v chains + transposes emitted (attention can
                start); ("q", sj, h) = head h's q chain + rope emitted."""
                proj_q.extend(proj_chain_units(sj, 0))       # k
                proj_q.extend(proj_chain_units(sj, 1))       # q0
                proj_q.extend(proj_chain_units(sj, 2))       # v
                proj_q.extend(vtp_units(sj))
                proj_q.append(("marker", ("tp", sj)))
                enqueue_q_chains(sj)

            def prolog_block0():
                """Block-0 k/q0/v chains interleaved at dt granularity so the
                PE consumes x/wqkv quarters as the startup DMAs land (the v
                chain borrows a PSUM bank from the idle "o" tag); q1-q3 ride
                the deferred queue, pulled in by attention(0)'s head-0 forces."""
                pps = {
                    0: ps.tile([HD, SB], F32, tag="pp", bufs=2, name="pp_0_0"),
                    1: ps.tile([HD, SB], F32, tag="pp", bufs=2, name="pp_0_1"),
                    2: ps.tile([HD, SB], F32, tag="o", bufs=2, name="pp_0_2"),
                }
                # chains run SEQUENTIALLY (q0, then k, then v) so q0's chain
                # stops ~7us earlier than a dt-interleave would allow and its
                # rope (the first score's gate) starts immediately; the k/v
                # chain matmuls then overlap the rope work on DVE/gpsimd
                for m in (1, 0, 2):
                    for dt in range(NT):
                        nc.tensor.matmul(
                            out=pps[m],
                            lhsT=wqkv[:, dt, m * HD : (m + 1) * HD],
                            rhs=xq_ap(0, dt),
                            start=(dt == 0),
                            stop=(dt == NT - 1),
                        )
                    if m == 1:
                        rope_emit(pps[1], 0, 1)
                # rope-k in two s-halves so kt tiles 0-1 are ready earlier
                for lo, hi in ((0, SB // 2), (SB // 2, SB)):
                    wd = hi - lo
                    ta = work.tile([H2, SB], F32, tag="ropeA")
                    tb = work.tile([H2, SB], F32, tag="ropeB")
                    nc.vector.tensor_mul(
                        out=ta[:, 0:wd], in0=pps[0][0:H2, lo:hi], in1=cost[:, lo:hi]
                    )
                    nc.vector.tensor_mul(
                        out=tb[:, 0:wd], in0=pps[0][H2:HD, lo:hi], in1=sint[:, lo:hi]
                    )
                    nc.gpsimd.tensor_sub(
                        out=kt[0:H2, lo:hi], in0=ta[:, 0:wd], in1=tb[:, 0:wd]
                    )
                    tc2 = work.tile([H2, SB], F32, tag="ropeA")
                    td = work.tile([H2, SB], F32, tag="ropeB")
                    nc.vector.tensor_mul(
                        out=tc2[:, 0:wd], in0=pps[0][0:H2, lo:hi], in1=sint[:, lo:hi]
                    )
                    nc.vector.tensor_mul(
                        out=td[:, 0:wd], in0=pps[0][H2:HD, lo:hi], in1=cost[:, lo:hi]
                    )
                    nc.vector.tensor_add(
                        out=kt[H2:HD, lo:hi], in0=tc2[:, 0:wd], in1=td[:, 0:wd]
                    )
                vt = work.tile([HD, SB], BF16, tag="vt")
                nc.scalar.copy(out=vt, in_=pps[2])
                vt_pending[0] = vt
                for kind, f in vtp_units(0):
                    f()
                passed.add(("tp", 0))
                enqueue_q_chains(0)
                # the k/q0 ropes take ~8us of serial DVE/gpsimd after the
                # chains stop; run the q1/q2 chains meanwhile so the PE
                # doesn't idle between prolog and attention(0)
                drain_until(("q", 0, 1))
                drain_until(("q", 0, 2))

            def append_wo_block(sj):
                for stl in range(4):
                    st = sj * 4 + stl
                    t0 = st * HD
                    for dj in range(NSJ):
                        state = {}
                        for hh in range(NH):
                            def f(hh=hh, dj=dj, st=st, t0=t0, state=state):
                                if hh == 0:
                                    state["ps_y"] = ps.tile(
                                        [HD, SB], F32, tag="pp", bufs=2,
                                        name=f"ps_y_{st}_{dj}",
                                    )
                                nc.tensor.matmul(
                                    out=state["ps_y"],
                                    lhsT=on_sb[:, hh, t0 : t0 + HD],
                                    rhs=wo_sb[:, hh, dj * SB : (dj + 1) * SB],
                                    start=(hh == 0),
                                    stop=(hh == NH - 1),
                                )
                                if hh == NH - 1:
                                    y_sb = work.tile(
                                        [HD, SB], BF16, tag="ysb", bufs=4,
                                        name=f"ysb_{st}_{dj}",
                                    )
                                    # scalar takes 3 of 4 copies (DVE is the
                                    # rope/acc engine); all y DMA issues ride
                                    # the idle sync engine (scalar issue cost
                                    # ~650ns each would eat exp headroom)
                                    if dj % 4 == 3:
                                        nc.vector.tensor_copy(y_sb, state["ps_y"])
                                    else:
                                        nc.scalar.copy(out=y_sb, in_=state["ps_y"])
                                    nc.sync.dma_start(
                                        out=y_d[t0 : t0 + HD, dj * SB : (dj + 1) * SB],
                                        in_=y_sb,
                                    )
                            wo_q.append(f)

            # ---- main loop ----
            prolog_block0()

            STUFF_RATE = {0: 4, 1: 5, 2: 4, 3: 3}
            WO_KEEP = {0: 64, 1: 128, 2: 128, 3: 0}

            for sj in range(NSJ):
                s0 = sj * SB
                if sj + 1 < NSJ:
                    enqueue_block_proj(sj + 1)

                nt = 4 * sj + 4  # causal: t-tiles 0..nt-1
                LOOKAHEAD = 4
                r = STUFF_RATE[sj]
                deferred_norm = [None]
                hstate = {}

                def emit_front(h, ti, hstate=hstate, sj=sj):
                    qts_, acc_ = hstate[h]["q"], hstate[h]["acc"]
                    kdiag = ti - 4 * sj
                    c0 = max(0, kdiag) * HD  # first valid column (diag band)
                    ps_s = ps.tile(
                        [HD, SB], F32, tag="s", bufs=3, name=f"s_{sj}_{h}_{ti}"
                    )
                    nc.tensor.matmul(
                        out=ps_s[:, c0:SB],
                        lhsT=kt[:, ti * HD : (ti + 1) * HD],
                        rhs=qts_[:, c0:SB],
                        start=True,
                        stop=True,
                    )
                    es = es_pool.tile(
                        [HD, SB], BF16, tag="es", bufs=8, name=f"es_{sj}_{h}_{ti}"
                    )
                    nc.scalar.activation(
                        out=es[:, c0:SB], in_=ps_s[:, c0:SB], func=Exp,
                        scale=float(SCALE),
                    )
                    if kdiag >= 0:
                        # triangular part: first HD valid columns; block 0 is
                        # rope-congested on gpsimd, so alternate with DVE there
                        eng = nc.vector if ti % 2 == 1 else nc.gpsimd
                        eng.tensor_mul(
                            out=es[:, c0 : c0 + HD],
                            in0=es[:, c0 : c0 + HD],
                            in1=tri,
                        )
                    if ti == 0:
                        hstate[h]["es0"] = es  # acc init fused into ti=1's add
                    elif ti == 1:
                        es0 = hstate[h]["es0"]
                        nc.vector.tensor_add(
                            out=acc_[:, c0:SB], in0=es0[:, c0:SB],
                            in1=es[:, c0:SB],
                        )
                        if c0 > 0:
                            nc.vector.tensor_copy(acc_[:, 0:c0], es0[:, 0:c0])
                    else:
                        nc.vector.tensor_add(
                            out=acc_[:, c0:SB], in0=acc_[:, c0:SB],
                            in1=es[:, c0:SB],
                        )
                    return (h, ti, es, c0)

                def emit_back(item, hstate=hstate, nt=nt):
                    h, ti, es, c0 = item
                    nc.tensor.matmul(
                        out=hstate[h]["o"][:, c0:SB],
                        lhsT=v_sb[:, ti, :],
                        rhs=es[:, c0:SB],
                        start=(ti == 0),
                        stop=(ti == nt - 1),
                    )

                def make_norm(h, hstate=hstate, sj=sj, s0=s0):
                    def norm_emit():
                        # den = colsum(acc), broadcast via all-ones stationary
                        ps_den = ps.tile(
                            [HD, SB], F32, tag="den", bufs=1, name=f"den_{sj}_{h}"
                        )
                        nc.tensor.matmul(
                            out=ps_den, lhsT=ones_sb, rhs=hstate[h]["acc"],
                            start=True, stop=True,
                        )
                        rb = work.tile([HD, SB], F32, tag="rb")
                        nc.vector.reciprocal_approx_fast(out=rb, in_=ps_den)
                        nc.vector.tensor_mul(
                            out=on_sb[:, h, s0 : s0 + SB], in0=hstate[h]["o"],
                            in1=rb,
                        )
                    return norm_emit

                # flat (h, ti) pipeline: the back stream lags LOOKAHEAD tiles
                # and crosses head boundaries, so head starts have no bubble
                pend = []
                drain_until(("tp", sj))
                for h in range(NH):
                    hstate[h] = {
                        "q": q_tiles[(sj, h)],
                        "o": ps.tile([HD, SB], F32, tag="o", bufs=2,
                                     name=f"o_{sj}_{h}"),
                        "acc": es_pool.tile([HD, SB], BF16, tag="acc", bufs=2,
                                            name=f"acc_{sj}_{h}"),
                    }
                    for ti in range(nt):
                        pend.append(emit_front(h, ti))
                        if len(pend) > LOOKAHEAD:
                            emit_back(pend.pop(0))
                        if ti == 3 and deferred_norm[0] is not None:
                            deferred_norm[0]()
                            deferred_norm[0] = None
                        for _ in range(r):
                            # the last block reserves wo units to bridge the
                            # final norm's latency (a PE idle there drops the
                            # clock p-state and slows the whole wo tail)
                            pop_one(wo_floor=12 if sj == NSJ - 1 else 0)
                        # pull the q chains through early: all three pop
                        # during head 0 (PE-dense clusters; their ropes
                        # pipeline on DVE one head ahead of consumption)
                        if h == 0 and ti in (0, 1, 2):
                            drain_until(("q", sj, ti + 1))
                        elif h >= 1 and h + 1 < NH and ti == 0:
                            drain_until(("q", sj, h + 1))
                    deferred_norm[0] = make_norm(h)
                while pend:
                    emit_back(pend.pop(0))

                # cover the last head's colsum latency with a few queue pops
                for _ in range(8):
                    pop_one(wo_floor=4 if sj == NSJ - 1 else 0)
                deferred_norm[0]()
                deferred_norm[0] = None

                append_wo_block(sj)
                # keep wo backlog to stuff later attention blocks; block 3's
                # own chains are the only tail
                while len(wo_q) > WO_KEEP[sj]:
                    wo_q.pop(0)()
            while pop_one():
                pass

    nc.compile()
    return nc


def _get_program():
    global _PROG
    if _PROG is None:
        _PROG = _build_program()
    return _PROG


def _make_in_maps(x, freqs_cos, freqs_sin, wq, wk, wv, wo):
    perm = np.concatenate([np.arange(0, HD, 2), np.arange(1, HD, 2)])  # even|odd

    costT = np.ascontiguousarray(np.asarray(freqs_cos, np.float32).T).astype(F16)
    sintT = np.ascontiguousarray(np.asarray(freqs_sin, np.float32).T).astype(F16)

    tt = np.arange(HD)[:, None]
    ss = np.arange(HD)[None, :]
    tri = (tt <= ss).astype(BF)  # lower-tri in [t, s]: valid iff t <= s
    ident = np.eye(HD, dtype=BF)
    ones = np.ones((HD, HD), dtype=BF)

    # permute q/k head-dim columns so rope pairs land on partition halves
    def permute_heads(w, n_heads):
        w = np.asarray(w, np.float32).reshape(D, n_heads, HD)
        return w[:, :, perm].reshape(D, n_heads * HD)

    wq_p = permute_heads(wq, N_HEADS)
    wk_p = permute_heads(wk, N_KV_HEADS)
    wv_ = np.asarray(wv, np.float32)
    wo_ = np.asarray(wo, np.float32)
    x_ = np.asarray(x, np.float32)

    in_maps = []
    for c in range(8):
        b, g = divmod(c, 4)
        # column order [k, q0, v, q1, q2, q3]: the first 384-col half feeds
        # the interleaved block-0 prolog; q heads then arrive in use order
        wq_g = wq_p[:, g * NH * HD : (g + 1) * NH * HD]
        wqkv = np.concatenate(
            [
                wk_p[:, g * HD : (g + 1) * HD],
                wq_g[:, 0:HD],
                wv_[:, g * HD : (g + 1) * HD],
                wq_g[:, HD:],
            ],
            axis=1,
        )
        in_maps.append(
            {
                "xt": np.ascontiguousarray(x_[b].T).astype(BF),
                "wqkv": np.ascontiguousarray(wqkv).astype(BF),
                "wo": np.ascontiguousarray(
                    wo_[g * NH * HD : (g + 1) * NH * HD, :]
                ).astype(BF),
                "cost": costT,
                "sint": sintT,
                "tri": tri,
                "ident": ident,
                "ones": ones,
            }
        )
    return in_maps


def run(x, freqs_cos, freqs_sin, wq, wk, wv, wo, trace=False):
    from concourse.bass_utils import run_bass_kernel_spmd

    nc = _get_program()
    in_maps = _make_in_maps(x, freqs_cos, freqs_sin, wq, wk, wv, wo)
    res = run_bass_kernel_spmd(nc, in_maps, list(range(8)), trace=trace)
    out = np.empty((B, S, D), dtype=np.float32)
    for b in range(B):
        acc = res.results[b * 4]["y"].astype(np.float32)
        for g in range(1, 4):
            acc = acc + res.results[b * 4 + g]["y"].astype(np.float32)
        out[b] = acc
    return out, res


def kernel(x, freqs_cos, freqs_sin, wq, wk, wv, wo):
    out, _ = run(x, freqs_cos, freqs_sin, wq, wk, wv, wo, trace=False)
    return out
